# revision 73
# baseline (speedup 1.0000x reference)
"""Multi-head causal attention on 8 Trainium2 NeuronCores.

Problem: B=2, T=2048, C=1024, H=16, HS=64 (fp32 in/out), causal mask.
Sharding: 8 cores = 2 batches x 4 head-groups (4 heads each); host sums the
4 per-batch partial output projections and adds the bias.

Optimizations vs the 208us fp32r baseline (measured ~157.8-158.8us across
runs, was 171.1us at the start of tuning; residual variance is HAM
clock-phase luck at kernel start, and a P0 thermal downclock to ~2.0GHz can
inflate runs ~19% until the chip idles a few minutes):
  - prologue de-serialized: only q0/k0 chains run before att(0,0); the v0-3
    chains sit at the filler-queue front and are pulled inside att(0,0)
    after its first scores (pulls=4), overlapping them with score/exp work
  - the first three attention blocks split their wide EXPs into 512-col
    halves so each AV chunk starts after only its half (those blocks have
    no proj fillers yet to hide the scores->exp->AV pipeline-fill latency)
  - vtc (AV stationary operand) padded 65->128 cols so every LDWEIGHTS is
    128-wide and qualifies for compiler-automatic fast-weight-load
  - all matmul operands bf16 (1 cyc/col on PE, same as fp32r; halves DMA+SBUF;
    measured absmax-rel err ~4e-3 vs 2e-2 gate)
  - ragged causal diagonal: the 4 diagonal ts-chunks of each tq-block compute
    only cols >= 128d (saves ~12% of scores/AV/exp); 128x128 triangle masks
  - score matmuls K=64 run 2-concurrent via PE row-tiling (auto tile_position
    from the 64/128 partition bases)
  - 3-queue input DMA tuned for the ~2.2us per-DMA completion latency and the
    4 semaphore slots per HWDGE queue: xtJ0 split sync/scalar (8 in flight),
    wq/wv/wp on gpsimd's 8-slot SWDGE queue, wk sync/scalar, then xtJ1 and
    merged xtJ2+3 on sync only (keeps scalar free for EXPs and gpsimd free
    for affine_selects once attention starts)
  - heartbeat matmuls (zero-accumulate into live PSUM groups) gated on
    arriving DMA chunks keep the PE HAM clock from re-throttling to 4/8
    through the DMA-paced prologue chains
  - tail: RESERVE proj tiles held back and emitted right after the last
    block's normalize chain (real work covering the serial recip/broadcast/
    mul window); last-block normalize reads pa directly and pipelines
    vector/gpsimd; tail proj PSUM->SBUF casts alternate vector/ACT (ACT is
    EXP-idle there; never put late-gated ops on the EXP queue mid-kernel -
    head-of-line blocking stalls AV)
  - merged schedule: attention blocks for both head-pairs interleaved with
    qkv/proj filler units so the PE never idles; proj spread through the
    second half (y writes overlap compute)
"""

import numpy as np

B, T, C, H, HS = 2, 2048, 1024, 16, 64
NCORES = 8
HPC = 4            # heads per core
NKC = C // 128     # contraction chunks (8)
NJ = T // 512      # tq blocks (4)
NTS = T // 128     # ts chunks (16)
NWARM = 8          # PE clock-ramp warmup matmuls

_NC_CACHE = {}


def _build_nc():
    if "nc" in _NC_CACHE:
        return _NC_CACHE["nc"]
    from contextlib import ExitStack
    import concourse.bass as bass
    from concourse import bacc, tile, mybir

    f32 = mybir.dt.float32
    bf16 = mybir.dt.bfloat16
    EXP = mybir.ActivationFunctionType.Exp
    COPYF = mybir.ActivationFunctionType.Copy

    nc = bacc.Bacc("TRN2", target_bir_lowering=False, debug=False,
                   enable_asserts=False, num_devices=NCORES)

    xT_d = nc.dram_tensor("xT", (C, T), bf16, kind="ExternalInput").ap()
    wq_d = nc.dram_tensor("wq_s", (C, HPC * HS), bf16, kind="ExternalInput").ap()
    wk_d = nc.dram_tensor("wk_s", (C, HPC * HS), bf16, kind="ExternalInput").ap()
    wv_d = nc.dram_tensor("wv_s", (C, HPC * HS), bf16, kind="ExternalInput").ap()
    wp_d = nc.dram_tensor("wp_s", (HPC * HS, C), bf16, kind="ExternalInput").ap()
    y_d = nc.dram_tensor("y", (T, C), bf16, kind="ExternalOutput").ap()

    scale = float(1.0 / np.sqrt(HS))

    with tile.TileContext(nc) as tc, ExitStack() as ctx:
        persist = ctx.enter_context(tc.tile_pool(name="persist", bufs=1))
        work = ctx.enter_context(tc.tile_pool(name="work", bufs=3))
        small = ctx.enter_context(tc.tile_pool(name="small", bufs=2))
        outp = ctx.enter_context(tc.tile_pool(name="outp", bufs=6))
        psp = ctx.enter_context(tc.tile_pool(name="psp", bufs=2, space="PSUM"))
        psaux = ctx.enter_context(tc.tile_pool(name="psaux", bufs=2, space="PSUM"))
        psatt = ctx.enter_context(tc.tile_pool(name="psatt", bufs=2, space="PSUM"))

        # ---- persistent SBUF tensors (all bf16) ----
        xt = [persist.tile([128, T], bf16, tag=f"xt{c}", name=f"xt{c}")
              for c in range(NKC)]
        # per-chunk weight tiles: a reader waits only its own chunk's DMA
        wq_sb = [persist.tile([128, 256], bf16, tag=f"wq{c}", name=f"wq{c}")
                 for c in range(NKC)]
        wk_sb = [persist.tile([128, 256], bf16, tag=f"wk{c}", name=f"wk{c}")
                 for c in range(NKC)]
        wv_sb = [persist.tile([128, 256], bf16, tag=f"wv{c}", name=f"wv{c}")
                 for c in range(NKC)]

        def wqs(c):
            return wq_sb[c]

        def wks(c):
            return wk_sb[c]

        def wvs(c):
            return wv_sb[c]
        wp_sb = persist.tile([128, 2, C], bf16, tag="wp")
        qT = [persist.tile([128, T], bf16, tag=f"qT{p}", name=f"qT{p}") for p in range(2)]
        kT = [persist.tile([128, T], bf16, tag=f"kT{p}", name=f"kT{p}") for p in range(2)]
        # 65 used cols (64 v-dims + ones for the denominator), padded to 128
        # so the AV LDWEIGHTS qualifies for compiler-automatic FWL
        vtc = persist.tile([128, NTS, HPC, 128], bf16, tag="vtc")
        attnT = [persist.tile([128, T], bf16, tag=f"attnT{p}", name=f"attnT{p}")
                 for p in range(2)]
        zeros = persist.tile([128, 512], bf16, tag="zeros")
        ones_t = persist.tile([128, NTS, HPC, 1], bf16, tag="ones")

        # ---- init (gpsimd memset starts earliest after boot; zeros gates
        # the PE warmups so it must be ready ASAP) ----
        nc.gpsimd.memset(zeros, 0.0)
        nc.vector.memset(ones_t, 1.0)
        nc.vector.memset(vtc[:, :, :, 65:128], 0.0)
        nc.vector.tensor_copy(out=vtc[:, :, :, 64:65], in_=ones_t)

        # ---- input DMAs, consumption order, 3 queues ----
        # HWDGE (sync evens / scalar odds): wq+xtJ0 interleaved, wk,
        #   xtJ1-J3 merged per chunk.  SWDGE (gpsimd): wv, wp, wp1lo.
        def _ld_w(eng, dst, src, c):
            eng.dma_start(out=dst[c], in_=src[c * 128:(c + 1) * 128, :])

        def _ld_x(eng, c, J):
            eng.dma_start(out=xt[c][:, 512 * J:512 * J + 512],
                          in_=xT_d[c * 128:(c + 1) * 128, 512 * J:512 * J + 512])

        # gpsimd's SWDGE queue has 8 semaphore slots (vs 4 on the HWDGE
        # queues) -> small weight loads go there; the 8 xtJ0 chunks split
        # across sync/scalar so all 8 are in flight at once
        for c in range(NKC):
            _ld_w(nc.gpsimd, wq_sb, wq_d, c)
        for c in range(NKC):
            _ld_x((nc.sync, nc.scalar)[c % 2], c, 0)
        for c in range(NKC):
            _ld_w(nc.gpsimd, wv_sb, wv_d, c)
        for c in range(NKC):
            _ld_w((nc.sync, nc.scalar)[c % 2], wk_sb, wk_d, c)
        nc.gpsimd.dma_start(out=wp_sb, in_=wp_d.rearrange("(k p) n -> p k n", p=128))
        # xtJ1 fine-grained sync/scalar; xtJ2+3 merged all on sync: keeps
        # scalar free for EXPs and gpsimd free for affine_selects from ~14us
        for c in range(NKC):
            _ld_x((nc.sync, nc.scalar)[c % 2], c, 1)
        for c in range(NKC):
            nc.sync.dma_start(out=xt[c][:, 1024:2048],
                              in_=xT_d[c * 128:(c + 1) * 128, 1024:2048])

        # ---- PE warmup (clock ramp during DMA) ----
        for i in range(NWARM):
            pw = psaux.tile([128, 512], f32, tag="aux", name=f"warm{i}")
            nc.tensor.matmul(pw, lhsT=zeros[:, 0:128], rhs=zeros,
                             start=True, stop=True)

        # ---- heartbeat helpers: keep HAM at 8/8 through stalls ----
        hbn = [0]

        def hb_unit(gate_ap):
            # standalone junk matmul in a fresh aux tile, gated on gate_ap
            hbn[0] += 1
            pw = psaux.tile([128, 512], f32, tag="aux", name=f"hbu{hbn[0]}")
            n = min(512, gate_ap.shape[-1])
            k = gate_ap.shape[0]
            nc.tensor.matmul(pw[:, 0:n], lhsT=zeros[0:k, 0:128],
                             rhs=gate_ap[:, 0:n], start=True, stop=True)

        def hb_in(ps_region, gate_ap, n=128):
            # accumulate-zero into a live accumulation region: numeric no-op
            n = min(n, ps_region.shape[-1], gate_ap.shape[-1])
            nc.tensor.matmul(ps_region[:, 0:n], lhsT=zeros[:, 0:128],
                             rhs=gate_ap[:, 0:n], start=False, stop=False,
                             skip_group_check=True)

        # ---------- compute unit emitters ----------
        # (PSUM->SBUF copies must be on DVE/ACT: GPSIMD cannot access PSUM)
        def _act_copy(out, in_):
            nc.scalar.activation(out=out, in_=in_, func=COPYF)

        class _Cp:
            def __init__(self, fn):
                self.tensor_copy = lambda out, in_: fn(out=out, in_=in_)

        cp_rot = [nc.vector, _Cp(_act_copy)]

        def qk_chain(pair, dst, w_of, J, eng_i, hbw=0):
            ps = psaux.tile([128, 512], f32, tag="aux", name=f"qk{id(dst)}_{pair}_{J}")
            for c in range(NKC):
                nc.tensor.matmul(ps, lhsT=w_of(c)[:, 128 * pair:128 * pair + 128],
                                 rhs=xt[c][:, 512 * J:512 * J + 512],
                                 start=(c == 0), stop=(c == NKC - 1))
                if hbw and 0 < c < NKC - 1:
                    for _ in range(hbw):
                        hb_in(ps, w_of(c))
            cp_rot[eng_i % 2].tensor_copy(out=dst[:, 512 * J:512 * J + 512], in_=ps)

        def v_chain(t, hbw=0):
            ps = psaux.tile([128, 512], f32, tag="aux", name=f"v_{t}")
            for c in range(NKC):
                nc.tensor.matmul(ps[:, 0:256], lhsT=xt[c][:, 128 * t:128 * t + 128],
                                 rhs=wvs(c), start=(c == 0), stop=(c == NKC - 1))
                if hbw and 0 < c < NKC - 1:
                    hb_in(ps[:, 0:256], wvs(c))
            cp_rot[t % 2].tensor_copy(
                out=vtc[:, t, :, 0:64],
                in_=ps[:, 0:256].rearrange("p (h x) -> p h x", x=64))

        tail_mode = [False]    # after last att block, proj can use the psp pool

        def proj_tile(m, n):
            if tail_mode[0] and m >= 12:
                py = psp.tile([128, 1024], f32, tag="s",
                              name=f"y_{m}_{n}")[:, 0:512]
            else:
                py = psaux.tile([128, 512], f32, tag="aux", name=f"y_{m}_{n}")
            for pair in range(2):
                nc.tensor.matmul(py, lhsT=attnT[pair][:, 128 * m:128 * m + 128],
                                 rhs=wp_sb[:, pair, 512 * n:512 * n + 512],
                                 start=(pair == 0), stop=(pair == 1))
            yo = outp.tile([128, 512], bf16, tag="yo")
            if tail_mode[0] and (m + n) % 2 == 0:
                # ACT is mostly idle in the tail; alternate with vector
                nc.scalar.activation(out=yo, in_=py, func=COPYF)
            else:
                nc.vector.tensor_copy(out=yo, in_=py)
            nc.sync.dma_start(out=y_d[128 * m:128 * m + 128, 512 * n:512 * n + 512],
                              in_=yo)

        # ---------- filler machinery ----------
        filler = []          # deque of (key, closure)
        emitted = set()

        # 8 units bridge the final normalize chain.  Measured both ways:
        # RESERVE=4 feeds (1,3)'s per-u bubbles but under-covers the
        # normalize window (a 4.3us gap appears at its end) and nets +4us;
        # the full 8 is the right trade.
        RESERVE = 8

        def pull(n):
            for _ in range(n):
                if filler and (tail_mode[0] or len(filler) > RESERVE):
                    k, f = filler.pop(0)
                    f()
                    emitted.add(k)

        def need(*keys):
            # selective: emit only the required units, leave the rest queued
            for k in keys:
                if k in emitted:
                    continue
                for idx, (fk, f) in enumerate(filler):
                    if fk == k:
                        filler.pop(idx)
                        f()
                        emitted.add(k)
                        break
                else:
                    raise RuntimeError(f"missing filler {k}")

        # ---------- attention block pair (both heads interleaved) ----------
        defer = []     # deferred finishers (prev block's normalize, proj adds)

        def att_pair(pair, J, pulls=1, last=False, split_exp=False):
            nch = 4 * J + 4
            # u-iterations: [(t, ss_off, N, qoff, pa_off, diag_stride), ...] x <=2
            us = []
            for i in range(2 * J):
                t0, t1 = 2 * i, 2 * i + 1
                us.append([(t0, 0, 512, 0, 0, 0), (t1, 512, 512, 0, 0, 0)])
            d0 = 4 * J
            us.append([(d0, 0, 512, 0, 0, 512), (d0 + 1, 512, 384, 128, 128, 512)])
            us.append([(d0 + 2, 0, 256, 256, 256, 256),
                       (d0 + 3, 256, 128, 384, 384, 256)])

            pa = [psatt.tile([128, 512], f32, tag="att", name=f"pa_{2*pair+hh}_{J}")
                  for hh in range(2)]
            pend = None
            for ui, u in enumerate(us):
                used = u[-1][1] + u[-1][2]
                dstr = u[0][5]
                et = work.tile([128, 2048], bf16, tag="et", bufs=3)
                for hh in range(2):
                    ss = psp.tile([128, 1024], f32, tag="s",
                                  name=f"ss_{2*pair+hh}_{J}_{u[0][0]}")
                    for (t, off, N, qoff, paoff, _) in u:
                        # K=64 contraction on the head's own partition range
                        nc.tensor.matmul(
                            ss[:, off:off + N],
                            lhsT=kT[pair][64 * hh:64 * hh + 64, 128 * t:128 * t + 128],
                            rhs=qT[pair][64 * hh:64 * hh + 64,
                                         512 * J + qoff:512 * J + qoff + N],
                            start=True, stop=True)
                    if split_exp and used > 512:
                        # early blocks have no proj fillers to hide the
                        # scores->exp->AV fill latency: halving the exp lets
                        # the first AV chunk start one half-exp earlier
                        nc.scalar.activation(
                            out=et[:, 1024 * hh:1024 * hh + 512],
                            in_=ss[:, 0:512], func=EXP, scale=scale)
                        nc.scalar.activation(
                            out=et[:, 1024 * hh + 512:1024 * hh + used],
                            in_=ss[:, 512:used], func=EXP, scale=scale)
                    else:
                        nc.scalar.activation(
                            out=et[:, 1024 * hh:1024 * hh + used],
                            in_=ss[:, 0:used], func=EXP, scale=scale)
                if dstr == 512:
                    # 4 causal triangles (2 heads x 2 diag chunks) evenly strided
                    sl = et.rearrange("p (d e) -> p d e", d=4)[:, :, 0:128]
                    nc.gpsimd.affine_select(
                        out=sl, in_=sl, compare_op=mybir.AluOpType.is_ge,
                        fill=0.0, base=0,
                        pattern=[[0, 4], [1, 128]], channel_multiplier=-1)
                elif dstr == 256:
                    for hh in range(2):
                        sl = et[:, 1024 * hh:1024 * hh + 512]
                        sl = sl.rearrange("p (d e) -> p d e", d=2)[:, :, 0:128]
                        nc.gpsimd.affine_select(
                            out=sl, in_=sl, compare_op=mybir.AluOpType.is_ge,
                            fill=0.0, base=0,
                            pattern=[[0, 2], [1, 128]], channel_multiplier=-1)
                if pend is not None:
                    pet, pu = pend
                    for hh in range(2):
                        for (t, off, N, qoff, paoff, _) in pu:
                            nc.tensor.matmul(pa[hh][:, paoff:paoff + N],
                                             lhsT=vtc[:, t, 2 * pair + hh, :],
                                             rhs=pet[:, 1024 * hh + off:1024 * hh + off + N],
                                             start=(t == 0), stop=False)
                pend = (et, u)
                pull(pulls)
                if ui == 0:
                    for fin in defer:
                        fin()
                    defer.clear()
            pet, pu = pend
            for hh in range(2):
                for (t, off, N, qoff, paoff, _) in pu:
                    nc.tensor.matmul(pa[hh][:, paoff:paoff + N],
                                     lhsT=vtc[:, t, 2 * pair + hh, :],
                                     rhs=pet[:, 1024 * hh + off:1024 * hh + off + N],
                                     start=(t == 0), stop=(t == nch - 1))

            # free the pa PSUM tiles immediately: copy denom row + value block
            # to SBUF now; the recip/broadcast/mul chain is deferred into the
            # next block so it never sits ahead of its critical gpsimd ops.
            # The last block has no successor: read pa directly and pipeline
            # vector/gpsimd; the reserved real proj tiles emitted right after
            # the chain keep the PE busy through it.
            if last:
                def finish():
                    s1 = small.tile([1, 512], f32, tag="lsums1")
                    nc.vector.tensor_copy(out=s1, in_=pa[1][64:65, :])
                    s0 = small.tile([1, 512], f32, tag="lsums0")
                    nc.vector.tensor_copy(out=s0, in_=pa[0][64:65, :])
                    rs1 = small.tile([1, 512], f32, tag="rsum")
                    nc.vector.reciprocal_approx_fast(out=rs1, in_=s1)
                    rs0 = small.tile([1, 512], f32, tag="rsum")
                    nc.vector.reciprocal_approx_fast(out=rs0, in_=s0)
                    rb1 = small.tile([64, 512], f32, tag="recip")
                    nc.gpsimd.partition_broadcast(rb1, rs1)
                    tmp = small.tile([64, 512], bf16, tag="tmp")
                    nc.vector.tensor_mul(tmp, pa[1][0:64, :], rb1)
                    rb0 = small.tile([64, 512], f32, tag="recip")
                    nc.gpsimd.partition_broadcast(rb0, rs0)
                    nc.gpsimd.dma_start(
                        out=attnT[pair][64:128, 512 * J:512 * J + 512], in_=tmp)
                    nc.vector.tensor_mul(
                        attnT[pair][0:64, 512 * J:512 * J + 512],
                        pa[0][0:64, :], rb0)
                return finish

            sums = [None, None]
            acc = [None, None]
            for hh in (1, 0):
                sums[hh] = small.tile([1, 512], f32, tag=f"sums{hh}",
                                      name=f"sums{hh}_{pair}_{J}")
                nc.vector.tensor_copy(out=sums[hh], in_=pa[hh][64:65, :])
                acc[hh] = small.tile([64, 512], f32, tag=f"acc{hh}",
                                     name=f"acc{hh}_{pair}_{J}")
                nc.vector.tensor_copy(out=acc[hh], in_=pa[hh][0:64, :])

            def finish():
                for hh in (1, 0):
                    rs = small.tile([1, 512], f32, tag="rsum")
                    nc.vector.reciprocal_approx_fast(out=rs, in_=sums[hh])
                    recip = small.tile([64, 512], f32, tag="recip")
                    nc.gpsimd.partition_broadcast(recip, rs)
                    if hh == 0:
                        nc.vector.tensor_mul(
                            attnT[pair][0:64, 512 * J:512 * J + 512],
                            acc[hh], recip)
                    else:
                        tmp = small.tile([64, 512], bf16, tag="tmp")
                        nc.vector.tensor_mul(tmp, acc[hh], recip)
                        nc.gpsimd.dma_start(
                            out=attnT[pair][64:128, 512 * J:512 * J + 512], in_=tmp)
            return finish

        # ---------- phase A: minimal serial prologue ----------
        # heartbeat-dense: hb_in fires right after each consumed chunk so the
        # PE never idles >~0.5us while DMA paces the first chains
        # v0-3 are NOT serial prologue: they go to the filler-queue front and
        # are pulled inside att(0,0) right after its first scores (pulls=4),
        # overlapping the v chains with score/exp of the first block.  The
        # first AV of (0,0) touches chunks t0,t1 and is emitted after u0's
        # pull, so dependency order stays correct.
        qk_chain(0, qT[0], wqs, 0, 0, hbw=2)
        hb_unit(xt[6][:, 0:512])
        hb_unit(xt[7][:, 0:512])
        qk_chain(0, kT[0], wks, 0, 1, hbw=2)
        hb_unit(wks(5))
        hb_unit(wks(7))

        # ---------- fillers in consumption order ----------
        def qk_unit(pair, dst, w_sb, J, key, eng_i):
            filler.append((key, lambda: qk_chain(pair, dst, w_sb, J, eng_i)))

        # sure-ready warmup units keep the PE clock ramped through the
        # DMA-bound early region
        def warm_unit(i):
            pw = psaux.tile([128, 512], f32, tag="aux", name=f"wf{i}")
            nc.tensor.matmul(pw, lhsT=zeros[:, 0:128], rhs=zeros,
                             start=True, stop=True)

        # v0-3 at the filler front: pulled by att(0,0)'s u0 (pulls=4) so they
        # run after the first scores instead of serializing the prologue
        for t in range(4):
            filler.append((("v", t), lambda t=t: v_chain(t)))
        for i in range(4):
            filler.append((("w", i), lambda i=i: warm_unit(i)))

        ei = 0
        for grp in range(1, NJ):
            # q0/k0 at J=grp ; q1/k1 at J=grp-1 ; v chunks for t-range
            qk_unit(0, qT[0], wqs, grp, ("q", 0, grp), ei); ei += 1
            qk_unit(0, kT[0], wks, grp, ("k", 0, grp), ei); ei += 1
            for t in range(4 * grp, 4 * grp + 4):
                filler.append((("v", t), lambda t=t: v_chain(t)))
            qk_unit(1, qT[1], wqs, grp - 1, ("q", 1, grp - 1), ei); ei += 1
            qk_unit(1, kT[1], wks, grp - 1, ("k", 1, grp - 1), ei); ei += 1
        qk_unit(1, qT[1], wqs, 3, ("q", 1, 3), ei); ei += 1
        qk_unit(1, kT[1], wks, 3, ("k", 1, 3), ei); ei += 1

        def add_proj(J):
            for m in range(4 * J, 4 * J + 4):
                for n in range(2):
                    filler.append(
                        (("proj", m, n), lambda m=m, n=n: proj_tile(m, n)))

        # ---------- merged attention schedule ----------
        def dep_att(pair, J):
            ks = [("q", pair, J), ("k", pair, J)] if (pair, J) != (0, 0) else []
            ks += [("v", t) for t in range(4, 4 * J + 4)]
            return ks

        order = [(0, 0), (0, 1), (1, 0), (0, 2), (1, 1), (0, 3), (1, 2), (1, 3)]
        for (pair, J) in order:
            need(*dep_att(pair, J))
            # split_exp: early blocks have no proj fillers yet; the last
            # block has none left (RESERVE) - both rely on shorter exp
            # latency to close their scores->exp->AV bubbles
            fin = att_pair(pair, J,
                           pulls=(4 if (pair, J) == (0, 0) else 1),
                           last=((pair, J) == order[-1]),
                           split_exp=((pair, J) in order[:3]
                                      or (pair, J) == order[-1]))
            defer.append(fin)
            if pair == 1:
                defer.append(lambda J=J: add_proj(J))
        tail_mode[0] = True
        for fin in defer:
            fin()
        defer.clear()
        pull(len(filler))

    nc.compile()
    _NC_CACHE["nc"] = nc
    return nc


def make_in_maps(x, wq, wk, wv, wproj):
    import ml_dtypes
    bf = ml_dtypes.bfloat16
    xTs = [np.ascontiguousarray(x[b].T).astype(bf) for b in range(B)]
    in_maps = []
    for core in range(NCORES):
        b, g = divmod(core, 4)
        hs = slice(4 * g, 4 * g + 4)
        in_maps.append({
            "xT": xTs[b],
            "wq_s": np.ascontiguousarray(
                wq[hs].transpose(1, 0, 2).reshape(C, HPC * HS)).astype(bf),
            "wk_s": np.ascontiguousarray(
                wk[hs].transpose(1, 0, 2).reshape(C, HPC * HS)).astype(bf),
            "wv_s": np.ascontiguousarray(
                wv[hs].transpose(1, 0, 2).reshape(C, HPC * HS)).astype(bf),
            "wp_s": np.ascontiguousarray(
                wproj[4 * g * HS:(4 * g + 4) * HS, :]).astype(bf),
        })
    return in_maps


def _assemble(results, bproj):
    y = np.zeros((B, T, C), dtype=np.float32)
    for core in range(NCORES):
        y[core // 4] += results[core]["y"].astype(np.float32)
    y += bproj.astype(np.float32)[None, None, :]
    return y


def _is_causal(attention_mask):
    tril = np.tril(np.ones((T, T), dtype=bool))
    return all(np.array_equal(attention_mask[b], tril) for b in range(B))


def _numpy_fallback(x, attention_mask, wq, wk, wv, wproj, bproj):
    x64 = x.astype(np.float32)
    q = np.einsum('btc,hcd->bhtd', x64, wq)
    k = np.einsum('btc,hcd->bhtd', x64, wk)
    v = np.einsum('btc,hcd->bhtd', x64, wv)
    wei = np.einsum('bhtd,bhsd->bhts', q, k) / np.sqrt(np.float32(HS))
    wei = np.where(attention_mask[:, None, :, :], wei, -np.inf)
    wei = wei - wei.max(axis=-1, keepdims=True)
    wei = np.exp(wei)
    wei = wei / wei.sum(axis=-1, keepdims=True)
    out = np.einsum('bhts,bhsd->bhtd', wei, v)
    out = out.transpose(0, 2, 1, 3).reshape(B, T, H * HS)
    return (out @ wproj + bproj).astype(np.float32)


def _install_ntff_hook():
    """Recreate the antenv.axon_hooks shim so trace=True works under axon."""
    import sys, types
    try:
        from antenv.axon_hooks import get_axon_ntff_profile_hook  # noqa
        return
    except ImportError:
        pass
    import antenv
    mod = types.ModuleType("antenv.axon_hooks")
    holder = [None]
    mod.set_axon_ntff_profile_hook = lambda h: holder.__setitem__(0, h)
    mod.get_axon_ntff_profile_hook = lambda: holder[0]
    sys.modules["antenv.axon_hooks"] = mod
    antenv.axon_hooks = mod
    if "/root/.axon_site" not in sys.path:
        sys.path.insert(0, "/root/.axon_site")
    from trn_agent_boot.trn_boot import _ntff_profile_via_ctypes
    mod.set_axon_ntff_profile_hook(_ntff_profile_via_ctypes("/opt/axon/libaxon_pjrt.so"))


def kernel(x, attention_mask, wq, wk, wv, wproj, bproj, _trace=False):
    x = np.asarray(x); attention_mask = np.asarray(attention_mask)
    wq = np.asarray(wq); wk = np.asarray(wk); wv = np.asarray(wv)
    wproj = np.asarray(wproj); bproj = np.asarray(bproj)

    if not _is_causal(attention_mask):
        return _numpy_fallback(x, attention_mask, wq, wk, wv, wproj, bproj)

    from concourse import bass_utils
    if _trace:
        _install_ntff_hook()
        bass_utils.upload_artifacts = lambda d: d
    nc = _build_nc()
    in_maps = make_in_maps(x, wq, wk, wv, wproj)
    res = bass_utils.run_bass_kernel_spmd(
        nc, in_maps, core_ids=list(range(NCORES)), trace=_trace)
    out = _assemble(res.results, bproj)
    if _trace:
        return out, res
    return out


# revision 75
# speedup vs baseline: 1.0235x; 1.0235x over previous
"""Multi-head causal attention on 8 Trainium2 NeuronCores.

Problem: B=2, T=2048, C=1024, H=16, HS=64 (fp32 in/out), causal mask.
Sharding: 8 cores = 2 batches x 4 head-groups (4 heads each); host sums the
4 per-batch partial output projections and adds the bias.

Optimizations vs the 208us fp32r baseline (measured ~157.8-158.8us across
runs, was 171.1us at the start of tuning; residual variance is HAM
clock-phase luck at kernel start, and a P0 thermal downclock to ~2.0GHz can
inflate runs ~19% until the chip idles a few minutes):
  - prologue de-serialized: only q0/k0 chains run before att(0,0); the v0-3
    chains sit at the filler-queue front and are pulled inside att(0,0)
    after its first scores (pulls=4), overlapping them with score/exp work
  - the first three attention blocks split their wide EXPs into 512-col
    halves so each AV chunk starts after only its half (those blocks have
    no proj fillers yet to hide the scores->exp->AV pipeline-fill latency)
  - vtc (AV stationary operand) padded 65->128 cols so every LDWEIGHTS is
    128-wide and qualifies for compiler-automatic fast-weight-load
  - all matmul operands bf16 (1 cyc/col on PE, same as fp32r; halves DMA+SBUF;
    measured absmax-rel err ~4e-3 vs 2e-2 gate)
  - ragged causal diagonal: the 4 diagonal ts-chunks of each tq-block compute
    only cols >= 128d (saves ~12% of scores/AV/exp); 128x128 triangle masks
  - score matmuls K=64 run 2-concurrent via PE row-tiling (auto tile_position
    from the 64/128 partition bases)
  - 3-queue input DMA tuned for the ~2.2us per-DMA completion latency and the
    4 semaphore slots per HWDGE queue: xtJ0 split sync/scalar (8 in flight),
    wq/wv/wp on gpsimd's 8-slot SWDGE queue, wk sync/scalar, then xtJ1 and
    merged xtJ2+3 on sync only (keeps scalar free for EXPs and gpsimd free
    for affine_selects once attention starts)
  - heartbeat matmuls (zero-accumulate into live PSUM groups) gated on
    arriving DMA chunks keep the PE HAM clock from re-throttling to 4/8
    through the DMA-paced prologue chains
  - tail: RESERVE proj tiles held back and emitted right after the last
    block's normalize chain (real work covering the serial recip/broadcast/
    mul window); last-block normalize reads pa directly and pipelines
    vector/gpsimd; tail proj PSUM->SBUF casts alternate vector/ACT (ACT is
    EXP-idle there; never put late-gated ops on the EXP queue mid-kernel -
    head-of-line blocking stalls AV)
  - merged schedule: attention blocks for both head-pairs interleaved with
    qkv/proj filler units so the PE never idles; proj spread through the
    second half (y writes overlap compute)
"""

import numpy as np

B, T, C, H, HS = 2, 2048, 1024, 16, 64
NCORES = 8
HPC = 4            # heads per core
NKC = C // 128     # contraction chunks (8)
NJ = T // 512      # tq blocks (4)
NTS = T // 128     # ts chunks (16)
NWARM = 8          # PE clock-ramp warmup matmuls

_NC_CACHE = {}


def _build_nc():
    if "nc" in _NC_CACHE:
        return _NC_CACHE["nc"]
    from contextlib import ExitStack
    import concourse.bass as bass
    from concourse import bacc, tile, mybir

    f32 = mybir.dt.float32
    bf16 = mybir.dt.bfloat16
    EXP = mybir.ActivationFunctionType.Exp
    COPYF = mybir.ActivationFunctionType.Copy

    nc = bacc.Bacc("TRN2", target_bir_lowering=False, debug=False,
                   enable_asserts=False, num_devices=NCORES)

    xT_d = nc.dram_tensor("xT", (C, T), bf16, kind="ExternalInput").ap()
    wq_d = nc.dram_tensor("wq_s", (C, HPC * HS), bf16, kind="ExternalInput").ap()
    wk_d = nc.dram_tensor("wk_s", (C, HPC * HS), bf16, kind="ExternalInput").ap()
    wv_d = nc.dram_tensor("wv_s", (C, HPC * HS), bf16, kind="ExternalInput").ap()
    wp_d = nc.dram_tensor("wp_s", (HPC * HS, C), bf16, kind="ExternalInput").ap()
    y_d = nc.dram_tensor("y", (T, C), bf16, kind="ExternalOutput").ap()

    scale = float(1.0 / np.sqrt(HS))

    with tile.TileContext(nc) as tc, ExitStack() as ctx:
        persist = ctx.enter_context(tc.tile_pool(name="persist", bufs=1))
        work = ctx.enter_context(tc.tile_pool(name="work", bufs=3))
        small = ctx.enter_context(tc.tile_pool(name="small", bufs=2))
        outp = ctx.enter_context(tc.tile_pool(name="outp", bufs=6))
        psp = ctx.enter_context(tc.tile_pool(name="psp", bufs=2, space="PSUM"))
        psaux = ctx.enter_context(tc.tile_pool(name="psaux", bufs=2, space="PSUM"))
        psatt = ctx.enter_context(tc.tile_pool(name="psatt", bufs=2, space="PSUM"))

        # ---- persistent SBUF tensors (all bf16) ----
        xt = [persist.tile([128, T], bf16, tag=f"xt{c}", name=f"xt{c}")
              for c in range(NKC)]
        # per-chunk weight tiles: a reader waits only its own chunk's DMA
        wq_sb = [persist.tile([128, 256], bf16, tag=f"wq{c}", name=f"wq{c}")
                 for c in range(NKC)]
        wk_sb = [persist.tile([128, 256], bf16, tag=f"wk{c}", name=f"wk{c}")
                 for c in range(NKC)]
        wv_sb = [persist.tile([128, 256], bf16, tag=f"wv{c}", name=f"wv{c}")
                 for c in range(NKC)]

        def wqs(c):
            return wq_sb[c]

        def wks(c):
            return wk_sb[c]

        def wvs(c):
            return wv_sb[c]
        wp_sb = persist.tile([128, 2, C], bf16, tag="wp")
        qT = [persist.tile([128, T], bf16, tag=f"qT{p}", name=f"qT{p}") for p in range(2)]
        kT = [persist.tile([128, T], bf16, tag=f"kT{p}", name=f"kT{p}") for p in range(2)]
        # 65 used cols (64 v-dims + ones for the denominator), padded to 128
        # so the AV LDWEIGHTS qualifies for compiler-automatic FWL
        vtc = persist.tile([128, NTS, HPC, 128], bf16, tag="vtc")
        attnT = [persist.tile([128, T], bf16, tag=f"attnT{p}", name=f"attnT{p}")
                 for p in range(2)]
        zeros = persist.tile([128, 512], bf16, tag="zeros")
        ones_t = persist.tile([128, NTS, HPC, 1], bf16, tag="ones")

        # ---- init (gpsimd memset starts earliest after boot; zeros gates
        # the PE warmups so it must be ready ASAP) ----
        nc.gpsimd.memset(zeros, 0.0)
        nc.vector.memset(ones_t, 1.0)
        nc.vector.memset(vtc[:, :, :, 65:128], 0.0)
        nc.vector.tensor_copy(out=vtc[:, :, :, 64:65], in_=ones_t)

        # ---- input DMAs, consumption order, 3 queues ----
        # HWDGE (sync evens / scalar odds): wq+xtJ0 interleaved, wk,
        #   xtJ1-J3 merged per chunk.  SWDGE (gpsimd): wv, wp, wp1lo.
        def _ld_w(eng, dst, src, c):
            eng.dma_start(out=dst[c], in_=src[c * 128:(c + 1) * 128, :])

        def _ld_x(eng, c, J):
            eng.dma_start(out=xt[c][:, 512 * J:512 * J + 512],
                          in_=xT_d[c * 128:(c + 1) * 128, 512 * J:512 * J + 512])

        # gpsimd's SWDGE queue has 8 semaphore slots (vs 4 on the HWDGE
        # queues) -> small weight loads go there; the 8 xtJ0 chunks split
        # across sync/scalar so all 8 are in flight at once
        for c in range(NKC):
            _ld_w(nc.gpsimd, wq_sb, wq_d, c)
        for c in range(NKC):
            _ld_x((nc.sync, nc.scalar)[c % 2], c, 0)
        for c in range(NKC):
            _ld_w(nc.gpsimd, wv_sb, wv_d, c)
        for c in range(NKC):
            _ld_w((nc.sync, nc.scalar)[c % 2], wk_sb, wk_d, c)
        nc.gpsimd.dma_start(out=wp_sb, in_=wp_d.rearrange("(k p) n -> p k n", p=128))
        # xtJ1 fine-grained sync/scalar; xtJ2+3 merged all on sync: keeps
        # scalar free for EXPs and gpsimd free for affine_selects from ~14us
        for c in range(NKC):
            _ld_x((nc.sync, nc.scalar)[c % 2], c, 1)
        for c in range(NKC):
            nc.sync.dma_start(out=xt[c][:, 1024:2048],
                              in_=xT_d[c * 128:(c + 1) * 128, 1024:2048])

        # ---- PE warmup (clock ramp during DMA) ----
        for i in range(NWARM):
            pw = psaux.tile([128, 512], f32, tag="aux", name=f"warm{i}")
            nc.tensor.matmul(pw, lhsT=zeros[:, 0:128], rhs=zeros,
                             start=True, stop=True)

        # ---- heartbeat helpers: keep HAM at 8/8 through stalls ----
        hbn = [0]

        def hb_unit(gate_ap):
            # standalone junk matmul in a fresh aux tile, gated on gate_ap
            hbn[0] += 1
            pw = psaux.tile([128, 512], f32, tag="aux", name=f"hbu{hbn[0]}")
            n = min(512, gate_ap.shape[-1])
            k = gate_ap.shape[0]
            nc.tensor.matmul(pw[:, 0:n], lhsT=zeros[0:k, 0:128],
                             rhs=gate_ap[:, 0:n], start=True, stop=True)

        def hb_in(ps_region, gate_ap, n=128):
            # accumulate-zero into a live accumulation region: numeric no-op
            n = min(n, ps_region.shape[-1], gate_ap.shape[-1])
            nc.tensor.matmul(ps_region[:, 0:n], lhsT=zeros[:, 0:128],
                             rhs=gate_ap[:, 0:n], start=False, stop=False,
                             skip_group_check=True)

        # ---------- compute unit emitters ----------
        # (PSUM->SBUF copies must be on DVE/ACT: GPSIMD cannot access PSUM)
        def _act_copy(out, in_):
            nc.scalar.activation(out=out, in_=in_, func=COPYF)

        class _Cp:
            def __init__(self, fn):
                self.tensor_copy = lambda out, in_: fn(out=out, in_=in_)

        cp_rot = [nc.vector, _Cp(_act_copy)]

        def qk_chain(pair, dst, w_of, J, eng_i, hbw=0):
            ps = psaux.tile([128, 512], f32, tag="aux", name=f"qk{id(dst)}_{pair}_{J}")
            for c in range(NKC):
                nc.tensor.matmul(ps, lhsT=w_of(c)[:, 128 * pair:128 * pair + 128],
                                 rhs=xt[c][:, 512 * J:512 * J + 512],
                                 start=(c == 0), stop=(c == NKC - 1))
                if hbw and 0 < c < NKC - 1:
                    for _ in range(hbw):
                        hb_in(ps, w_of(c))
            cp_rot[eng_i % 2].tensor_copy(out=dst[:, 512 * J:512 * J + 512], in_=ps)

        def v_chain(t, hbw=0):
            ps = psaux.tile([128, 512], f32, tag="aux", name=f"v_{t}")
            for c in range(NKC):
                nc.tensor.matmul(ps[:, 0:256], lhsT=xt[c][:, 128 * t:128 * t + 128],
                                 rhs=wvs(c), start=(c == 0), stop=(c == NKC - 1))
                if hbw and 0 < c < NKC - 1:
                    hb_in(ps[:, 0:256], wvs(c))
            cp_rot[t % 2].tensor_copy(
                out=vtc[:, t, :, 0:64],
                in_=ps[:, 0:256].rearrange("p (h x) -> p h x", x=64))

        tail_mode = [False]    # after last att block, proj can use the psp pool

        def proj_tile(m, n):
            if tail_mode[0] and m >= 12:
                py = psp.tile([128, 1024], f32, tag="s",
                              name=f"y_{m}_{n}")[:, 0:512]
            else:
                py = psaux.tile([128, 512], f32, tag="aux", name=f"y_{m}_{n}")
            for pair in range(2):
                nc.tensor.matmul(py, lhsT=attnT[pair][:, 128 * m:128 * m + 128],
                                 rhs=wp_sb[:, pair, 512 * n:512 * n + 512],
                                 start=(pair == 0), stop=(pair == 1))
            yo = outp.tile([128, 512], bf16, tag="yo")
            if tail_mode[0] and (m + n) % 2 == 0:
                # ACT is mostly idle in the tail; alternate with vector
                nc.scalar.activation(out=yo, in_=py, func=COPYF)
            else:
                nc.vector.tensor_copy(out=yo, in_=py)
            nc.sync.dma_start(out=y_d[128 * m:128 * m + 128, 512 * n:512 * n + 512],
                              in_=yo)

        # ---------- filler machinery ----------
        filler = []          # deque of (key, closure)
        emitted = set()

        # 8 units bridge the final normalize chain.  Measured both ways:
        # RESERVE=4 feeds (1,3)'s per-u bubbles but under-covers the
        # normalize window (a 4.3us gap appears at its end) and nets +4us;
        # the full 8 is the right trade.
        RESERVE = 8

        def pull(n):
            for _ in range(n):
                if filler and (tail_mode[0] or len(filler) > RESERVE):
                    k, f = filler.pop(0)
                    f()
                    emitted.add(k)

        def need(*keys):
            # selective: emit only the required units, leave the rest queued
            for k in keys:
                if k in emitted:
                    continue
                for idx, (fk, f) in enumerate(filler):
                    if fk == k:
                        filler.pop(idx)
                        f()
                        emitted.add(k)
                        break
                else:
                    raise RuntimeError(f"missing filler {k}")

        # ---------- attention block pair (both heads interleaved) ----------
        defer = []     # deferred finishers (prev block's normalize, proj adds)

        def att_pair(pair, J, pulls=1, last=False, split_exp=False):
            nch = 4 * J + 4
            # u-iterations: [(t, ss_off, N, qoff, pa_off, diag_stride), ...] x <=2
            us = []
            for i in range(2 * J):
                t0, t1 = 2 * i, 2 * i + 1
                us.append([(t0, 0, 512, 0, 0, 0), (t1, 512, 512, 0, 0, 0)])
            d0 = 4 * J
            us.append([(d0, 0, 512, 0, 0, 512), (d0 + 1, 512, 384, 128, 128, 512)])
            us.append([(d0 + 2, 0, 256, 256, 256, 256),
                       (d0 + 3, 256, 128, 384, 384, 256)])

            pa = [psatt.tile([128, 512], f32, tag="att", name=f"pa_{2*pair+hh}_{J}")
                  for hh in range(2)]
            pend = None
            for ui, u in enumerate(us):
                used = u[-1][1] + u[-1][2]
                dstr = u[0][5]
                et = work.tile([128, 2048], bf16, tag="et", bufs=3)
                for hh in range(2):
                    ss = psp.tile([128, 1024], f32, tag="s",
                                  name=f"ss_{2*pair+hh}_{J}_{u[0][0]}")
                    for (t, off, N, qoff, paoff, _) in u:
                        # K=64 contraction on the head's own partition range
                        nc.tensor.matmul(
                            ss[:, off:off + N],
                            lhsT=kT[pair][64 * hh:64 * hh + 64, 128 * t:128 * t + 128],
                            rhs=qT[pair][64 * hh:64 * hh + 64,
                                         512 * J + qoff:512 * J + qoff + N],
                            start=True, stop=True)
                    if split_exp and used > 512:
                        # early blocks have no proj fillers to hide the
                        # scores->exp->AV fill latency: halving the exp lets
                        # the first AV chunk start one half-exp earlier
                        nc.scalar.activation(
                            out=et[:, 1024 * hh:1024 * hh + 512],
                            in_=ss[:, 0:512], func=EXP, scale=scale)
                        nc.scalar.activation(
                            out=et[:, 1024 * hh + 512:1024 * hh + used],
                            in_=ss[:, 512:used], func=EXP, scale=scale)
                    else:
                        nc.scalar.activation(
                            out=et[:, 1024 * hh:1024 * hh + used],
                            in_=ss[:, 0:used], func=EXP, scale=scale)
                if dstr == 512:
                    # 2 causal triangles per head, split per head so each
                    # head's AV gates only on its own exp+affine (a single
                    # d=4 op would make AV-h0 wait for h1's exp too)
                    for hh in range(2):
                        sl = et[:, 1024 * hh:1024 * hh + 1024]
                        sl = sl.rearrange("p (d e) -> p d e", d=2)[:, :, 0:128]
                        nc.gpsimd.affine_select(
                            out=sl, in_=sl, compare_op=mybir.AluOpType.is_ge,
                            fill=0.0, base=0,
                            pattern=[[0, 2], [1, 128]], channel_multiplier=-1)
                elif dstr == 256:
                    for hh in range(2):
                        sl = et[:, 1024 * hh:1024 * hh + 512]
                        sl = sl.rearrange("p (d e) -> p d e", d=2)[:, :, 0:128]
                        nc.gpsimd.affine_select(
                            out=sl, in_=sl, compare_op=mybir.AluOpType.is_ge,
                            fill=0.0, base=0,
                            pattern=[[0, 2], [1, 128]], channel_multiplier=-1)
                if pend is not None:
                    pet, pu = pend
                    for hh in range(2):
                        for (t, off, N, qoff, paoff, _) in pu:
                            nc.tensor.matmul(pa[hh][:, paoff:paoff + N],
                                             lhsT=vtc[:, t, 2 * pair + hh, :],
                                             rhs=pet[:, 1024 * hh + off:1024 * hh + off + N],
                                             start=(t == 0), stop=False)
                pend = (et, u)
                pull(pulls)
                if ui == 0:
                    for fin in defer:
                        fin()
                    defer.clear()
            pet, pu = pend
            for hh in range(2):
                for (t, off, N, qoff, paoff, _) in pu:
                    nc.tensor.matmul(pa[hh][:, paoff:paoff + N],
                                     lhsT=vtc[:, t, 2 * pair + hh, :],
                                     rhs=pet[:, 1024 * hh + off:1024 * hh + off + N],
                                     start=(t == 0), stop=(t == nch - 1))

            # free the pa PSUM tiles immediately: copy denom row + value block
            # to SBUF now; the recip/broadcast/mul chain is deferred into the
            # next block so it never sits ahead of its critical gpsimd ops.
            # The last block has no successor: read pa directly and pipeline
            # vector/gpsimd; the reserved real proj tiles emitted right after
            # the chain keep the PE busy through it.
            if last:
                def finish():
                    s1 = small.tile([1, 512], f32, tag="lsums1")
                    nc.vector.tensor_copy(out=s1, in_=pa[1][64:65, :])
                    s0 = small.tile([1, 512], f32, tag="lsums0")
                    nc.vector.tensor_copy(out=s0, in_=pa[0][64:65, :])
                    rs1 = small.tile([1, 512], f32, tag="rsum")
                    nc.vector.reciprocal_approx_fast(out=rs1, in_=s1)
                    rs0 = small.tile([1, 512], f32, tag="rsum")
                    nc.vector.reciprocal_approx_fast(out=rs0, in_=s0)
                    rb1 = small.tile([64, 512], f32, tag="recip")
                    nc.gpsimd.partition_broadcast(rb1, rs1)
                    tmp = small.tile([64, 512], bf16, tag="tmp")
                    nc.vector.tensor_mul(tmp, pa[1][0:64, :], rb1)
                    rb0 = small.tile([64, 512], f32, tag="recip")
                    nc.gpsimd.partition_broadcast(rb0, rs0)
                    nc.gpsimd.dma_start(
                        out=attnT[pair][64:128, 512 * J:512 * J + 512], in_=tmp)
                    nc.vector.tensor_mul(
                        attnT[pair][0:64, 512 * J:512 * J + 512],
                        pa[0][0:64, :], rb0)
                return finish

            sums = [None, None]
            acc = [None, None]
            for hh in (1, 0):
                sums[hh] = small.tile([1, 512], f32, tag=f"sums{hh}",
                                      name=f"sums{hh}_{pair}_{J}")
                nc.vector.tensor_copy(out=sums[hh], in_=pa[hh][64:65, :])
                acc[hh] = small.tile([64, 512], f32, tag=f"acc{hh}",
                                     name=f"acc{hh}_{pair}_{J}")
                nc.vector.tensor_copy(out=acc[hh], in_=pa[hh][0:64, :])

            def finish():
                for hh in (1, 0):
                    rs = small.tile([1, 512], f32, tag="rsum")
                    nc.vector.reciprocal_approx_fast(out=rs, in_=sums[hh])
                    recip = small.tile([64, 512], f32, tag="recip")
                    nc.gpsimd.partition_broadcast(recip, rs)
                    if hh == 0:
                        nc.vector.tensor_mul(
                            attnT[pair][0:64, 512 * J:512 * J + 512],
                            acc[hh], recip)
                    else:
                        tmp = small.tile([64, 512], bf16, tag="tmp")
                        nc.vector.tensor_mul(tmp, acc[hh], recip)
                        nc.gpsimd.dma_start(
                            out=attnT[pair][64:128, 512 * J:512 * J + 512], in_=tmp)
            return finish

        # ---------- phase A: minimal serial prologue ----------
        # heartbeat-dense: hb_in fires right after each consumed chunk so the
        # PE never idles >~0.5us while DMA paces the first chains
        # v0-3 are NOT serial prologue: they go to the filler-queue front and
        # are pulled inside att(0,0) right after its first scores (pulls=4),
        # overlapping the v chains with score/exp of the first block.  The
        # first AV of (0,0) touches chunks t0,t1 and is emitted after u0's
        # pull, so dependency order stays correct.
        qk_chain(0, qT[0], wqs, 0, 0, hbw=2)
        hb_unit(xt[6][:, 0:512])
        hb_unit(xt[7][:, 0:512])
        qk_chain(0, kT[0], wks, 0, 1, hbw=2)
        hb_unit(wks(5))
        hb_unit(wks(7))

        # ---------- fillers in consumption order ----------
        def qk_unit(pair, dst, w_sb, J, key, eng_i):
            filler.append((key, lambda: qk_chain(pair, dst, w_sb, J, eng_i)))

        # sure-ready warmup units keep the PE clock ramped through the
        # DMA-bound early region
        def warm_unit(i):
            pw = psaux.tile([128, 512], f32, tag="aux", name=f"wf{i}")
            nc.tensor.matmul(pw, lhsT=zeros[:, 0:128], rhs=zeros,
                             start=True, stop=True)

        # v0-3 at the filler front: pulled by att(0,0)'s u0 (pulls=4) so they
        # run after the first scores instead of serializing the prologue
        for t in range(4):
            filler.append((("v", t), lambda t=t: v_chain(t)))
        for i in range(4):
            filler.append((("w", i), lambda i=i: warm_unit(i)))

        ei = 0
        for grp in range(1, NJ):
            # q0/k0 at J=grp ; q1/k1 at J=grp-1 ; v chunks for t-range
            qk_unit(0, qT[0], wqs, grp, ("q", 0, grp), ei); ei += 1
            qk_unit(0, kT[0], wks, grp, ("k", 0, grp), ei); ei += 1
            for t in range(4 * grp, 4 * grp + 4):
                filler.append((("v", t), lambda t=t: v_chain(t)))
            qk_unit(1, qT[1], wqs, grp - 1, ("q", 1, grp - 1), ei); ei += 1
            qk_unit(1, kT[1], wks, grp - 1, ("k", 1, grp - 1), ei); ei += 1
        qk_unit(1, qT[1], wqs, 3, ("q", 1, 3), ei); ei += 1
        qk_unit(1, kT[1], wks, 3, ("k", 1, 3), ei); ei += 1

        def add_proj(J):
            for m in range(4 * J, 4 * J + 4):
                for n in range(2):
                    filler.append(
                        (("proj", m, n), lambda m=m, n=n: proj_tile(m, n)))

        # ---------- merged attention schedule ----------
        def dep_att(pair, J):
            ks = [("q", pair, J), ("k", pair, J)] if (pair, J) != (0, 0) else []
            ks += [("v", t) for t in range(4, 4 * J + 4)]
            return ks

        order = [(0, 0), (0, 1), (1, 0), (0, 2), (1, 1), (0, 3), (1, 2), (1, 3)]
        for (pair, J) in order:
            need(*dep_att(pair, J))
            # split_exp only for the first three blocks (no proj fillers yet
            # to hide the scores->exp->AV fill).  Splitting (1,3)'s exps was
            # measured at +9us: the extra ACT instructions push its final AV
            # and the whole normalize window out.
            fin = att_pair(pair, J,
                           pulls=(4 if (pair, J) == (0, 0) else 1),
                           last=((pair, J) == order[-1]),
                           split_exp=((pair, J) in order[:3]))
            defer.append(fin)
            if pair == 1:
                defer.append(lambda J=J: add_proj(J))
        tail_mode[0] = True
        for fin in defer:
            fin()
        defer.clear()
        pull(len(filler))

    nc.compile()
    _NC_CACHE["nc"] = nc
    return nc


def make_in_maps(x, wq, wk, wv, wproj):
    import ml_dtypes
    bf = ml_dtypes.bfloat16
    xTs = [np.ascontiguousarray(x[b].T).astype(bf) for b in range(B)]
    in_maps = []
    for core in range(NCORES):
        b, g = divmod(core, 4)
        hs = slice(4 * g, 4 * g + 4)
        in_maps.append({
            "xT": xTs[b],
            "wq_s": np.ascontiguousarray(
                wq[hs].transpose(1, 0, 2).reshape(C, HPC * HS)).astype(bf),
            "wk_s": np.ascontiguousarray(
                wk[hs].transpose(1, 0, 2).reshape(C, HPC * HS)).astype(bf),
            "wv_s": np.ascontiguousarray(
                wv[hs].transpose(1, 0, 2).reshape(C, HPC * HS)).astype(bf),
            "wp_s": np.ascontiguousarray(
                wproj[4 * g * HS:(4 * g + 4) * HS, :]).astype(bf),
        })
    return in_maps


def _assemble(results, bproj):
    y = np.zeros((B, T, C), dtype=np.float32)
    for core in range(NCORES):
        y[core // 4] += results[core]["y"].astype(np.float32)
    y += bproj.astype(np.float32)[None, None, :]
    return y


def _is_causal(attention_mask):
    tril = np.tril(np.ones((T, T), dtype=bool))
    return all(np.array_equal(attention_mask[b], tril) for b in range(B))


def _numpy_fallback(x, attention_mask, wq, wk, wv, wproj, bproj):
    x64 = x.astype(np.float32)
    q = np.einsum('btc,hcd->bhtd', x64, wq)
    k = np.einsum('btc,hcd->bhtd', x64, wk)
    v = np.einsum('btc,hcd->bhtd', x64, wv)
    wei = np.einsum('bhtd,bhsd->bhts', q, k) / np.sqrt(np.float32(HS))
    wei = np.where(attention_mask[:, None, :, :], wei, -np.inf)
    wei = wei - wei.max(axis=-1, keepdims=True)
    wei = np.exp(wei)
    wei = wei / wei.sum(axis=-1, keepdims=True)
    out = np.einsum('bhts,bhsd->bhtd', wei, v)
    out = out.transpose(0, 2, 1, 3).reshape(B, T, H * HS)
    return (out @ wproj + bproj).astype(np.float32)


def _install_ntff_hook():
    """Recreate the antenv.axon_hooks shim so trace=True works under axon."""
    import sys, types
    try:
        from antenv.axon_hooks import get_axon_ntff_profile_hook  # noqa
        return
    except ImportError:
        pass
    import antenv
    mod = types.ModuleType("antenv.axon_hooks")
    holder = [None]
    mod.set_axon_ntff_profile_hook = lambda h: holder.__setitem__(0, h)
    mod.get_axon_ntff_profile_hook = lambda: holder[0]
    sys.modules["antenv.axon_hooks"] = mod
    antenv.axon_hooks = mod
    if "/root/.axon_site" not in sys.path:
        sys.path.insert(0, "/root/.axon_site")
    from trn_agent_boot.trn_boot import _ntff_profile_via_ctypes
    mod.set_axon_ntff_profile_hook(_ntff_profile_via_ctypes("/opt/axon/libaxon_pjrt.so"))


def kernel(x, attention_mask, wq, wk, wv, wproj, bproj, _trace=False):
    x = np.asarray(x); attention_mask = np.asarray(attention_mask)
    wq = np.asarray(wq); wk = np.asarray(wk); wv = np.asarray(wv)
    wproj = np.asarray(wproj); bproj = np.asarray(bproj)

    if not _is_causal(attention_mask):
        return _numpy_fallback(x, attention_mask, wq, wk, wv, wproj, bproj)

    from concourse import bass_utils
    if _trace:
        _install_ntff_hook()
        bass_utils.upload_artifacts = lambda d: d
    nc = _build_nc()
    in_maps = make_in_maps(x, wq, wk, wv, wproj)
    res = bass_utils.run_bass_kernel_spmd(
        nc, in_maps, core_ids=list(range(NCORES)), trace=_trace)
    out = _assemble(res.results, bproj)
    if _trace:
        return out, res
    return out


# revision 76
# speedup vs baseline: 1.0584x; 1.0341x over previous
"""Multi-head causal attention on 8 Trainium2 NeuronCores.

Problem: B=2, T=2048, C=1024, H=16, HS=64 (fp32 in/out), causal mask.
Sharding: 8 cores = 2 batches x 4 head-groups (4 heads each); host sums the
4 per-batch partial output projections and adds the bias.

Optimizations vs the 208us fp32r baseline (measured ~157.8-158.8us across
runs, was 171.1us at the start of tuning; residual variance is HAM
clock-phase luck at kernel start, and a P0 thermal downclock to ~2.0GHz can
inflate runs ~19% until the chip idles a few minutes):
  - prologue de-serialized: only q0/k0 chains run before att(0,0); the v0-3
    chains sit at the filler-queue front and are pulled inside att(0,0)
    after its first scores (pulls=4), overlapping them with score/exp work
  - the first three attention blocks split their wide EXPs into 512-col
    halves so each AV chunk starts after only its half (those blocks have
    no proj fillers yet to hide the scores->exp->AV pipeline-fill latency)
  - vtc (AV stationary operand) padded 65->128 cols so every LDWEIGHTS is
    128-wide and qualifies for compiler-automatic fast-weight-load
  - all matmul operands bf16 (1 cyc/col on PE, same as fp32r; halves DMA+SBUF;
    measured absmax-rel err ~4e-3 vs 2e-2 gate)
  - ragged causal diagonal: the 4 diagonal ts-chunks of each tq-block compute
    only cols >= 128d (saves ~12% of scores/AV/exp); 128x128 triangle masks
  - score matmuls K=64 run 2-concurrent via PE row-tiling (auto tile_position
    from the 64/128 partition bases)
  - 3-queue input DMA tuned for the ~2.2us per-DMA completion latency and the
    4 semaphore slots per HWDGE queue: xtJ0 split sync/scalar (8 in flight),
    wq/wv/wp on gpsimd's 8-slot SWDGE queue, wk sync/scalar, then xtJ1 and
    merged xtJ2+3 on sync only (keeps scalar free for EXPs and gpsimd free
    for affine_selects once attention starts)
  - heartbeat matmuls (zero-accumulate into live PSUM groups) gated on
    arriving DMA chunks keep the PE HAM clock from re-throttling to 4/8
    through the DMA-paced prologue chains
  - tail: RESERVE proj tiles held back and emitted right after the last
    block's normalize chain (real work covering the serial recip/broadcast/
    mul window); last-block normalize reads pa directly and pipelines
    vector/gpsimd; tail proj PSUM->SBUF casts alternate vector/ACT (ACT is
    EXP-idle there; never put late-gated ops on the EXP queue mid-kernel -
    head-of-line blocking stalls AV)
  - merged schedule: attention blocks for both head-pairs interleaved with
    qkv/proj filler units so the PE never idles; proj spread through the
    second half (y writes overlap compute)
"""

import numpy as np

B, T, C, H, HS = 2, 2048, 1024, 16, 64
NCORES = 8
HPC = 4            # heads per core
NKC = C // 128     # contraction chunks (8)
NJ = T // 512      # tq blocks (4)
NTS = T // 128     # ts chunks (16)
NWARM = 8          # PE clock-ramp warmup matmuls

_NC_CACHE = {}


def _build_nc():
    if "nc" in _NC_CACHE:
        return _NC_CACHE["nc"]
    from contextlib import ExitStack
    import concourse.bass as bass
    from concourse import bacc, tile, mybir

    f32 = mybir.dt.float32
    bf16 = mybir.dt.bfloat16
    EXP = mybir.ActivationFunctionType.Exp
    COPYF = mybir.ActivationFunctionType.Copy

    nc = bacc.Bacc("TRN2", target_bir_lowering=False, debug=False,
                   enable_asserts=False, num_devices=NCORES)

    xT_d = nc.dram_tensor("xT", (C, T), bf16, kind="ExternalInput").ap()
    wq_d = nc.dram_tensor("wq_s", (C, HPC * HS), bf16, kind="ExternalInput").ap()
    wk_d = nc.dram_tensor("wk_s", (C, HPC * HS), bf16, kind="ExternalInput").ap()
    wv_d = nc.dram_tensor("wv_s", (C, HPC * HS), bf16, kind="ExternalInput").ap()
    wp_d = nc.dram_tensor("wp_s", (HPC * HS, C), bf16, kind="ExternalInput").ap()
    y_d = nc.dram_tensor("y", (T, C), bf16, kind="ExternalOutput").ap()

    scale = float(1.0 / np.sqrt(HS))

    with tile.TileContext(nc) as tc, ExitStack() as ctx:
        persist = ctx.enter_context(tc.tile_pool(name="persist", bufs=1))
        work = ctx.enter_context(tc.tile_pool(name="work", bufs=3))
        small = ctx.enter_context(tc.tile_pool(name="small", bufs=2))
        outp = ctx.enter_context(tc.tile_pool(name="outp", bufs=6))
        psp = ctx.enter_context(tc.tile_pool(name="psp", bufs=2, space="PSUM"))
        psaux = ctx.enter_context(tc.tile_pool(name="psaux", bufs=2, space="PSUM"))
        psatt = ctx.enter_context(tc.tile_pool(name="psatt", bufs=2, space="PSUM"))

        # ---- persistent SBUF tensors (all bf16) ----
        xt = [persist.tile([128, T], bf16, tag=f"xt{c}", name=f"xt{c}")
              for c in range(NKC)]
        # per-chunk weight tiles: a reader waits only its own chunk's DMA
        wq_sb = [persist.tile([128, 256], bf16, tag=f"wq{c}", name=f"wq{c}")
                 for c in range(NKC)]
        wk_sb = [persist.tile([128, 256], bf16, tag=f"wk{c}", name=f"wk{c}")
                 for c in range(NKC)]
        wv_sb = [persist.tile([128, 256], bf16, tag=f"wv{c}", name=f"wv{c}")
                 for c in range(NKC)]

        def wqs(c):
            return wq_sb[c]

        def wks(c):
            return wk_sb[c]

        def wvs(c):
            return wv_sb[c]
        wp_sb = persist.tile([128, 2, C], bf16, tag="wp")
        qT = [persist.tile([128, T], bf16, tag=f"qT{p}", name=f"qT{p}") for p in range(2)]
        kT = [persist.tile([128, T], bf16, tag=f"kT{p}", name=f"kT{p}") for p in range(2)]
        # 65 used cols (64 v-dims + ones for the denominator), padded to 128
        # so the AV LDWEIGHTS qualifies for compiler-automatic FWL
        vtc = persist.tile([128, NTS, HPC, 128], bf16, tag="vtc")
        attnT = [persist.tile([128, T], bf16, tag=f"attnT{p}", name=f"attnT{p}")
                 for p in range(2)]
        zeros = persist.tile([128, 512], bf16, tag="zeros")
        ones_t = persist.tile([128, NTS, HPC, 1], bf16, tag="ones")

        # ---- init (gpsimd memset starts earliest after boot; zeros gates
        # the PE warmups so it must be ready ASAP) ----
        nc.gpsimd.memset(zeros, 0.0)
        nc.vector.memset(ones_t, 1.0)
        nc.vector.memset(vtc[:, :, :, 65:128], 0.0)
        nc.vector.tensor_copy(out=vtc[:, :, :, 64:65], in_=ones_t)

        # ---- input DMAs, consumption order, 3 queues ----
        # HWDGE (sync evens / scalar odds): wq+xtJ0 interleaved, wk,
        #   xtJ1-J3 merged per chunk.  SWDGE (gpsimd): wv, wp, wp1lo.
        def _ld_w(eng, dst, src, c):
            eng.dma_start(out=dst[c], in_=src[c * 128:(c + 1) * 128, :])

        def _ld_x(eng, c, J):
            eng.dma_start(out=xt[c][:, 512 * J:512 * J + 512],
                          in_=xT_d[c * 128:(c + 1) * 128, 512 * J:512 * J + 512])

        # gpsimd's SWDGE queue has 8 semaphore slots (vs 4 on the HWDGE
        # queues) -> small weight loads go there; the 8 xtJ0 chunks split
        # across sync/scalar so all 8 are in flight at once
        for c in range(NKC):
            _ld_w(nc.gpsimd, wq_sb, wq_d, c)
        for c in range(NKC):
            _ld_x((nc.sync, nc.scalar)[c % 2], c, 0)
        for c in range(NKC):
            _ld_w(nc.gpsimd, wv_sb, wv_d, c)
        for c in range(NKC):
            _ld_w((nc.sync, nc.scalar)[c % 2], wk_sb, wk_d, c)
        nc.gpsimd.dma_start(out=wp_sb, in_=wp_d.rearrange("(k p) n -> p k n", p=128))
        # xtJ1 fine-grained sync/scalar; xtJ2+3 merged all on sync: keeps
        # scalar free for EXPs and gpsimd free for affine_selects from ~14us
        for c in range(NKC):
            _ld_x((nc.sync, nc.scalar)[c % 2], c, 1)
        for c in range(NKC):
            nc.sync.dma_start(out=xt[c][:, 1024:2048],
                              in_=xT_d[c * 128:(c + 1) * 128, 1024:2048])

        # ---- PE warmup (clock ramp during DMA) ----
        for i in range(NWARM):
            pw = psaux.tile([128, 512], f32, tag="aux", name=f"warm{i}")
            nc.tensor.matmul(pw, lhsT=zeros[:, 0:128], rhs=zeros,
                             start=True, stop=True)

        # ---- heartbeat helpers: keep HAM at 8/8 through stalls ----
        hbn = [0]

        def hb_unit(gate_ap):
            # standalone junk matmul in a fresh aux tile, gated on gate_ap
            hbn[0] += 1
            pw = psaux.tile([128, 512], f32, tag="aux", name=f"hbu{hbn[0]}")
            n = min(512, gate_ap.shape[-1])
            k = gate_ap.shape[0]
            nc.tensor.matmul(pw[:, 0:n], lhsT=zeros[0:k, 0:128],
                             rhs=gate_ap[:, 0:n], start=True, stop=True)

        def hb_in(ps_region, gate_ap, n=128):
            # accumulate-zero into a live accumulation region: numeric no-op
            n = min(n, ps_region.shape[-1], gate_ap.shape[-1])
            nc.tensor.matmul(ps_region[:, 0:n], lhsT=zeros[:, 0:128],
                             rhs=gate_ap[:, 0:n], start=False, stop=False,
                             skip_group_check=True)

        # ---------- compute unit emitters ----------
        # (PSUM->SBUF copies must be on DVE/ACT: GPSIMD cannot access PSUM)
        def _act_copy(out, in_):
            nc.scalar.activation(out=out, in_=in_, func=COPYF)

        class _Cp:
            def __init__(self, fn):
                self.tensor_copy = lambda out, in_: fn(out=out, in_=in_)

        cp_rot = [nc.vector, _Cp(_act_copy)]

        def qk_chain(pair, dst, w_of, J, eng_i, hbw=0):
            ps = psaux.tile([128, 512], f32, tag="aux", name=f"qk{id(dst)}_{pair}_{J}")
            for c in range(NKC):
                nc.tensor.matmul(ps, lhsT=w_of(c)[:, 128 * pair:128 * pair + 128],
                                 rhs=xt[c][:, 512 * J:512 * J + 512],
                                 start=(c == 0), stop=(c == NKC - 1))
                if hbw and 0 < c < NKC - 1:
                    for _ in range(hbw):
                        hb_in(ps, w_of(c))
            cp_rot[eng_i % 2].tensor_copy(out=dst[:, 512 * J:512 * J + 512], in_=ps)

        def v_chain(t, hbw=0):
            ps = psaux.tile([128, 512], f32, tag="aux", name=f"v_{t}")
            for c in range(NKC):
                nc.tensor.matmul(ps[:, 0:256], lhsT=xt[c][:, 128 * t:128 * t + 128],
                                 rhs=wvs(c), start=(c == 0), stop=(c == NKC - 1))
                if hbw and 0 < c < NKC - 1:
                    hb_in(ps[:, 0:256], wvs(c))
            cp_rot[t % 2].tensor_copy(
                out=vtc[:, t, :, 0:64],
                in_=ps[:, 0:256].rearrange("p (h x) -> p h x", x=64))

        tail_mode = [False]    # after last att block, proj can use the psp pool

        def proj_tile(m, n):
            if tail_mode[0] and m >= 12:
                py = psp.tile([128, 1024], f32, tag="s",
                              name=f"y_{m}_{n}")[:, 0:512]
            else:
                py = psaux.tile([128, 512], f32, tag="aux", name=f"y_{m}_{n}")
            for pair in range(2):
                nc.tensor.matmul(py, lhsT=attnT[pair][:, 128 * m:128 * m + 128],
                                 rhs=wp_sb[:, pair, 512 * n:512 * n + 512],
                                 start=(pair == 0), stop=(pair == 1))
            yo = outp.tile([128, 512], bf16, tag="yo")
            if tail_mode[0] and (m + n) % 2 == 0:
                # ACT is mostly idle in the tail; alternate with vector
                nc.scalar.activation(out=yo, in_=py, func=COPYF)
            else:
                nc.vector.tensor_copy(out=yo, in_=py)
            nc.sync.dma_start(out=y_d[128 * m:128 * m + 128, 512 * n:512 * n + 512],
                              in_=yo)

        # ---------- filler machinery ----------
        filler = []          # deque of (key, closure)
        emitted = set()

        # 8 units bridge the final normalize chain.  Measured both ways:
        # RESERVE=4 feeds (1,3)'s per-u bubbles but under-covers the
        # normalize window (a 4.3us gap appears at its end) and nets +4us;
        # the full 8 is the right trade.
        RESERVE = 8

        def pull(n):
            for _ in range(n):
                if filler and (tail_mode[0] or len(filler) > RESERVE):
                    k, f = filler.pop(0)
                    f()
                    emitted.add(k)

        def need(*keys):
            # selective: emit only the required units, leave the rest queued
            for k in keys:
                if k in emitted:
                    continue
                for idx, (fk, f) in enumerate(filler):
                    if fk == k:
                        filler.pop(idx)
                        f()
                        emitted.add(k)
                        break
                else:
                    raise RuntimeError(f"missing filler {k}")

        # ---------- attention block pair (both heads interleaved) ----------
        defer = []     # deferred finishers (prev block's normalize, proj adds)

        def att_pair(pair, J, pulls=1, last=False, split_exp=False):
            nch = 4 * J + 4
            # u-iterations: [(t, ss_off, N, qoff, pa_off, diag_stride), ...] x <=2
            us = []
            for i in range(2 * J):
                t0, t1 = 2 * i, 2 * i + 1
                us.append([(t0, 0, 512, 0, 0, 0), (t1, 512, 512, 0, 0, 0)])
            d0 = 4 * J
            us.append([(d0, 0, 512, 0, 0, 512), (d0 + 1, 512, 384, 128, 128, 512)])
            us.append([(d0 + 2, 0, 256, 256, 256, 256),
                       (d0 + 3, 256, 128, 384, 384, 256)])

            pa = [psatt.tile([128, 512], f32, tag="att", name=f"pa_{2*pair+hh}_{J}")
                  for hh in range(2)]
            pend = None
            for ui, u in enumerate(us):
                used = u[-1][1] + u[-1][2]
                dstr = u[0][5]
                et = work.tile([128, 2048], bf16, tag="et", bufs=3)
                for hh in range(2):
                    ss = psp.tile([128, 1024], f32, tag="s",
                                  name=f"ss_{2*pair+hh}_{J}_{u[0][0]}")
                    for (t, off, N, qoff, paoff, _) in u:
                        # K=64 contraction on the head's own partition range
                        nc.tensor.matmul(
                            ss[:, off:off + N],
                            lhsT=kT[pair][64 * hh:64 * hh + 64, 128 * t:128 * t + 128],
                            rhs=qT[pair][64 * hh:64 * hh + 64,
                                         512 * J + qoff:512 * J + qoff + N],
                            start=True, stop=True)
                    if split_exp and used > 512:
                        # early blocks have no proj fillers to hide the
                        # scores->exp->AV fill latency: halving the exp lets
                        # the first AV chunk start one half-exp earlier
                        nc.scalar.activation(
                            out=et[:, 1024 * hh:1024 * hh + 512],
                            in_=ss[:, 0:512], func=EXP, scale=scale)
                        nc.scalar.activation(
                            out=et[:, 1024 * hh + 512:1024 * hh + used],
                            in_=ss[:, 512:used], func=EXP, scale=scale)
                    else:
                        nc.scalar.activation(
                            out=et[:, 1024 * hh:1024 * hh + used],
                            in_=ss[:, 0:used], func=EXP, scale=scale)
                if dstr == 512:
                    # 4 causal triangles (2 heads x 2 diag chunks) evenly
                    # strided, in ONE gpsimd op.  A per-head split (finer AV
                    # gating) was measured at +4us: the doubled gpsimd
                    # instruction count costs more in fixed per-op overhead
                    # on the contended gpsimd queue than the refinement saves.
                    sl = et.rearrange("p (d e) -> p d e", d=4)[:, :, 0:128]
                    nc.gpsimd.affine_select(
                        out=sl, in_=sl, compare_op=mybir.AluOpType.is_ge,
                        fill=0.0, base=0,
                        pattern=[[0, 4], [1, 128]], channel_multiplier=-1)
                elif dstr == 256:
                    for hh in range(2):
                        sl = et[:, 1024 * hh:1024 * hh + 512]
                        sl = sl.rearrange("p (d e) -> p d e", d=2)[:, :, 0:128]
                        nc.gpsimd.affine_select(
                            out=sl, in_=sl, compare_op=mybir.AluOpType.is_ge,
                            fill=0.0, base=0,
                            pattern=[[0, 2], [1, 128]], channel_multiplier=-1)
                if pend is not None:
                    pet, pu = pend
                    for hh in range(2):
                        for (t, off, N, qoff, paoff, _) in pu:
                            nc.tensor.matmul(pa[hh][:, paoff:paoff + N],
                                             lhsT=vtc[:, t, 2 * pair + hh, :],
                                             rhs=pet[:, 1024 * hh + off:1024 * hh + off + N],
                                             start=(t == 0), stop=False)
                pend = (et, u)
                pull(pulls)
                if ui == 0:
                    for fin in defer:
                        fin()
                    defer.clear()
            pet, pu = pend
            for hh in range(2):
                for (t, off, N, qoff, paoff, _) in pu:
                    nc.tensor.matmul(pa[hh][:, paoff:paoff + N],
                                     lhsT=vtc[:, t, 2 * pair + hh, :],
                                     rhs=pet[:, 1024 * hh + off:1024 * hh + off + N],
                                     start=(t == 0), stop=(t == nch - 1))

            # free the pa PSUM tiles immediately: copy denom row + value block
            # to SBUF now; the recip/broadcast/mul chain is deferred into the
            # next block so it never sits ahead of its critical gpsimd ops.
            # The last block has no successor: read pa directly and pipeline
            # vector/gpsimd; the reserved real proj tiles emitted right after
            # the chain keep the PE busy through it.
            if last:
                def finish():
                    s1 = small.tile([1, 512], f32, tag="lsums1")
                    nc.vector.tensor_copy(out=s1, in_=pa[1][64:65, :])
                    s0 = small.tile([1, 512], f32, tag="lsums0")
                    nc.vector.tensor_copy(out=s0, in_=pa[0][64:65, :])
                    rs1 = small.tile([1, 512], f32, tag="rsum")
                    nc.vector.reciprocal_approx_fast(out=rs1, in_=s1)
                    rs0 = small.tile([1, 512], f32, tag="rsum")
                    nc.vector.reciprocal_approx_fast(out=rs0, in_=s0)
                    rb1 = small.tile([64, 512], f32, tag="recip")
                    nc.gpsimd.partition_broadcast(rb1, rs1)
                    tmp = small.tile([64, 512], bf16, tag="tmp")
                    nc.vector.tensor_mul(tmp, pa[1][0:64, :], rb1)
                    rb0 = small.tile([64, 512], f32, tag="recip")
                    nc.gpsimd.partition_broadcast(rb0, rs0)
                    nc.gpsimd.dma_start(
                        out=attnT[pair][64:128, 512 * J:512 * J + 512], in_=tmp)
                    nc.vector.tensor_mul(
                        attnT[pair][0:64, 512 * J:512 * J + 512],
                        pa[0][0:64, :], rb0)
                return finish

            sums = [None, None]
            acc = [None, None]
            for hh in (1, 0):
                sums[hh] = small.tile([1, 512], f32, tag=f"sums{hh}",
                                      name=f"sums{hh}_{pair}_{J}")
                nc.vector.tensor_copy(out=sums[hh], in_=pa[hh][64:65, :])
                acc[hh] = small.tile([64, 512], f32, tag=f"acc{hh}",
                                     name=f"acc{hh}_{pair}_{J}")
                nc.vector.tensor_copy(out=acc[hh], in_=pa[hh][0:64, :])

            def finish():
                for hh in (1, 0):
                    rs = small.tile([1, 512], f32, tag="rsum")
                    nc.vector.reciprocal_approx_fast(out=rs, in_=sums[hh])
                    recip = small.tile([64, 512], f32, tag="recip")
                    nc.gpsimd.partition_broadcast(recip, rs)
                    if hh == 0:
                        nc.vector.tensor_mul(
                            attnT[pair][0:64, 512 * J:512 * J + 512],
                            acc[hh], recip)
                    else:
                        tmp = small.tile([64, 512], bf16, tag="tmp")
                        nc.vector.tensor_mul(tmp, acc[hh], recip)
                        nc.gpsimd.dma_start(
                            out=attnT[pair][64:128, 512 * J:512 * J + 512], in_=tmp)
            return finish

        # ---------- phase A: minimal serial prologue ----------
        # heartbeat-dense: hb_in fires right after each consumed chunk so the
        # PE never idles >~0.5us while DMA paces the first chains
        # v0-3 are NOT serial prologue: they go to the filler-queue front and
        # are pulled inside att(0,0) right after its first scores (pulls=4),
        # overlapping the v chains with score/exp of the first block.  The
        # first AV of (0,0) touches chunks t0,t1 and is emitted after u0's
        # pull, so dependency order stays correct.
        qk_chain(0, qT[0], wqs, 0, 0, hbw=2)
        hb_unit(xt[6][:, 0:512])
        hb_unit(xt[7][:, 0:512])
        qk_chain(0, kT[0], wks, 0, 1, hbw=2)
        hb_unit(wks(5))
        hb_unit(wks(7))

        # ---------- fillers in consumption order ----------
        def qk_unit(pair, dst, w_sb, J, key, eng_i):
            filler.append((key, lambda: qk_chain(pair, dst, w_sb, J, eng_i)))

        # sure-ready warmup units keep the PE clock ramped through the
        # DMA-bound early region
        def warm_unit(i):
            pw = psaux.tile([128, 512], f32, tag="aux", name=f"wf{i}")
            nc.tensor.matmul(pw, lhsT=zeros[:, 0:128], rhs=zeros,
                             start=True, stop=True)

        # v0-3 at the filler front: pulled by att(0,0)'s u0 (pulls=4) so they
        # run after the first scores instead of serializing the prologue
        for t in range(4):
            filler.append((("v", t), lambda t=t: v_chain(t)))
        for i in range(4):
            filler.append((("w", i), lambda i=i: warm_unit(i)))

        ei = 0
        for grp in range(1, NJ):
            # q0/k0 at J=grp ; q1/k1 at J=grp-1 ; v chunks for t-range
            qk_unit(0, qT[0], wqs, grp, ("q", 0, grp), ei); ei += 1
            qk_unit(0, kT[0], wks, grp, ("k", 0, grp), ei); ei += 1
            for t in range(4 * grp, 4 * grp + 4):
                filler.append((("v", t), lambda t=t: v_chain(t)))
            qk_unit(1, qT[1], wqs, grp - 1, ("q", 1, grp - 1), ei); ei += 1
            qk_unit(1, kT[1], wks, grp - 1, ("k", 1, grp - 1), ei); ei += 1
        qk_unit(1, qT[1], wqs, 3, ("q", 1, 3), ei); ei += 1
        qk_unit(1, kT[1], wks, 3, ("k", 1, 3), ei); ei += 1

        def add_proj(J):
            for m in range(4 * J, 4 * J + 4):
                for n in range(2):
                    filler.append(
                        (("proj", m, n), lambda m=m, n=n: proj_tile(m, n)))

        # ---------- merged attention schedule ----------
        def dep_att(pair, J):
            ks = [("q", pair, J), ("k", pair, J)] if (pair, J) != (0, 0) else []
            ks += [("v", t) for t in range(4, 4 * J + 4)]
            return ks

        order = [(0, 0), (0, 1), (1, 0), (0, 2), (1, 1), (0, 3), (1, 2), (1, 3)]
        for (pair, J) in order:
            need(*dep_att(pair, J))
            # split_exp only for the first three blocks (no proj fillers yet
            # to hide the scores->exp->AV fill).  Splitting (1,3)'s exps was
            # measured at +9us: the extra ACT instructions push its final AV
            # and the whole normalize window out.
            fin = att_pair(pair, J,
                           pulls=(4 if (pair, J) == (0, 0) else 1),
                           last=((pair, J) == order[-1]),
                           split_exp=((pair, J) in order[:3]))
            defer.append(fin)
            if pair == 1:
                defer.append(lambda J=J: add_proj(J))
        tail_mode[0] = True
        for fin in defer:
            fin()
        defer.clear()
        pull(len(filler))

    nc.compile()
    _NC_CACHE["nc"] = nc
    return nc


def make_in_maps(x, wq, wk, wv, wproj):
    import ml_dtypes
    bf = ml_dtypes.bfloat16
    xTs = [np.ascontiguousarray(x[b].T).astype(bf) for b in range(B)]
    in_maps = []
    for core in range(NCORES):
        b, g = divmod(core, 4)
        hs = slice(4 * g, 4 * g + 4)
        in_maps.append({
            "xT": xTs[b],
            "wq_s": np.ascontiguousarray(
                wq[hs].transpose(1, 0, 2).reshape(C, HPC * HS)).astype(bf),
            "wk_s": np.ascontiguousarray(
                wk[hs].transpose(1, 0, 2).reshape(C, HPC * HS)).astype(bf),
            "wv_s": np.ascontiguousarray(
                wv[hs].transpose(1, 0, 2).reshape(C, HPC * HS)).astype(bf),
            "wp_s": np.ascontiguousarray(
                wproj[4 * g * HS:(4 * g + 4) * HS, :]).astype(bf),
        })
    return in_maps


def _assemble(results, bproj):
    y = np.zeros((B, T, C), dtype=np.float32)
    for core in range(NCORES):
        y[core // 4] += results[core]["y"].astype(np.float32)
    y += bproj.astype(np.float32)[None, None, :]
    return y


def _is_causal(attention_mask):
    tril = np.tril(np.ones((T, T), dtype=bool))
    return all(np.array_equal(attention_mask[b], tril) for b in range(B))


def _numpy_fallback(x, attention_mask, wq, wk, wv, wproj, bproj):
    x64 = x.astype(np.float32)
    q = np.einsum('btc,hcd->bhtd', x64, wq)
    k = np.einsum('btc,hcd->bhtd', x64, wk)
    v = np.einsum('btc,hcd->bhtd', x64, wv)
    wei = np.einsum('bhtd,bhsd->bhts', q, k) / np.sqrt(np.float32(HS))
    wei = np.where(attention_mask[:, None, :, :], wei, -np.inf)
    wei = wei - wei.max(axis=-1, keepdims=True)
    wei = np.exp(wei)
    wei = wei / wei.sum(axis=-1, keepdims=True)
    out = np.einsum('bhts,bhsd->bhtd', wei, v)
    out = out.transpose(0, 2, 1, 3).reshape(B, T, H * HS)
    return (out @ wproj + bproj).astype(np.float32)


def _install_ntff_hook():
    """Recreate the antenv.axon_hooks shim so trace=True works under axon."""
    import sys, types
    try:
        from antenv.axon_hooks import get_axon_ntff_profile_hook  # noqa
        return
    except ImportError:
        pass
    import antenv
    mod = types.ModuleType("antenv.axon_hooks")
    holder = [None]
    mod.set_axon_ntff_profile_hook = lambda h: holder.__setitem__(0, h)
    mod.get_axon_ntff_profile_hook = lambda: holder[0]
    sys.modules["antenv.axon_hooks"] = mod
    antenv.axon_hooks = mod
    if "/root/.axon_site" not in sys.path:
        sys.path.insert(0, "/root/.axon_site")
    from trn_agent_boot.trn_boot import _ntff_profile_via_ctypes
    mod.set_axon_ntff_profile_hook(_ntff_profile_via_ctypes("/opt/axon/libaxon_pjrt.so"))


def kernel(x, attention_mask, wq, wk, wv, wproj, bproj, _trace=False):
    x = np.asarray(x); attention_mask = np.asarray(attention_mask)
    wq = np.asarray(wq); wk = np.asarray(wk); wv = np.asarray(wv)
    wproj = np.asarray(wproj); bproj = np.asarray(bproj)

    if not _is_causal(attention_mask):
        return _numpy_fallback(x, attention_mask, wq, wk, wv, wproj, bproj)

    from concourse import bass_utils
    if _trace:
        _install_ntff_hook()
        bass_utils.upload_artifacts = lambda d: d
    nc = _build_nc()
    in_maps = make_in_maps(x, wq, wk, wv, wproj)
    res = bass_utils.run_bass_kernel_spmd(
        nc, in_maps, core_ids=list(range(NCORES)), trace=_trace)
    out = _assemble(res.results, bproj)
    if _trace:
        return out, res
    return out


# revision 78
# speedup vs baseline: 1.0618x; 1.0033x over previous
"""Multi-head causal attention on 8 Trainium2 NeuronCores.

Problem: B=2, T=2048, C=1024, H=16, HS=64 (fp32 in/out), causal mask.
Sharding: 8 cores = 2 batches x 4 head-groups (4 heads each); host sums the
4 per-batch partial output projections and adds the bias.

Optimizations vs the 208us fp32r baseline (best measured 157467ns; cool-chip
band ~157.5-159us, was 171.1us at the start of tuning; residual variance is
HAM clock-phase luck at kernel start, and a P0 thermal downclock to ~2.0GHz
can inflate runs ~17-19% until the chip idles a few minutes):
  - prologue de-serialized: only q0/k0 chains run before att(0,0); the v0-3
    chains sit at the filler-queue front and are pulled inside att(0,0)
    after its first scores (pulls=4), overlapping them with score/exp work
  - the first three attention blocks split their wide EXPs into 512-col
    halves so each AV chunk starts after only its half (those blocks have
    no proj fillers yet to hide the scores->exp->AV pipeline-fill latency)
  - vtc (AV stationary operand) padded 65->128 cols so every LDWEIGHTS is
    128-wide and qualifies for compiler-automatic fast-weight-load
  - all matmul operands bf16 (1 cyc/col on PE, same as fp32r; halves DMA+SBUF;
    measured absmax-rel err ~4e-3 vs 2e-2 gate)
  - ragged causal diagonal: the 4 diagonal ts-chunks of each tq-block compute
    only cols >= 128d (saves ~12% of scores/AV/exp); 128x128 triangle masks
  - score matmuls K=64 run 2-concurrent via PE row-tiling (auto tile_position
    from the 64/128 partition bases)
  - 3-queue input DMA tuned for the ~2.2us per-DMA completion latency and the
    4 semaphore slots per HWDGE queue: xtJ0 split sync/scalar (8 in flight),
    wq/wv/wp on gpsimd's 8-slot SWDGE queue, wk sync/scalar, then xtJ1 and
    merged xtJ2+3 on sync only (keeps scalar free for EXPs and gpsimd free
    for affine_selects once attention starts)
  - heartbeat matmuls (zero-accumulate into live PSUM groups) gated on
    arriving DMA chunks keep the PE HAM clock from re-throttling to 4/8
    through the DMA-paced prologue chains
  - tail: RESERVE proj tiles held back and emitted right after the last
    block's normalize chain (real work covering the serial recip/broadcast/
    mul window); last-block normalize reads pa directly and pipelines
    vector/gpsimd; tail proj PSUM->SBUF casts alternate vector/ACT (ACT is
    EXP-idle there; never put late-gated ops on the EXP queue mid-kernel -
    head-of-line blocking stalls AV)
  - merged schedule: attention blocks for both head-pairs interleaved with
    qkv/proj filler units so the PE never idles; proj spread through the
    second half (y writes overlap compute)
"""

import numpy as np

B, T, C, H, HS = 2, 2048, 1024, 16, 64
NCORES = 8
HPC = 4            # heads per core
NKC = C // 128     # contraction chunks (8)
NJ = T // 512      # tq blocks (4)
NTS = T // 128     # ts chunks (16)
NWARM = 8          # PE clock-ramp warmup matmuls

_NC_CACHE = {}


def _build_nc():
    if "nc" in _NC_CACHE:
        return _NC_CACHE["nc"]
    from contextlib import ExitStack
    import concourse.bass as bass
    from concourse import bacc, tile, mybir

    f32 = mybir.dt.float32
    bf16 = mybir.dt.bfloat16
    EXP = mybir.ActivationFunctionType.Exp
    COPYF = mybir.ActivationFunctionType.Copy

    nc = bacc.Bacc("TRN2", target_bir_lowering=False, debug=False,
                   enable_asserts=False, num_devices=NCORES)

    xT_d = nc.dram_tensor("xT", (C, T), bf16, kind="ExternalInput").ap()
    wq_d = nc.dram_tensor("wq_s", (C, HPC * HS), bf16, kind="ExternalInput").ap()
    wk_d = nc.dram_tensor("wk_s", (C, HPC * HS), bf16, kind="ExternalInput").ap()
    wv_d = nc.dram_tensor("wv_s", (C, HPC * HS), bf16, kind="ExternalInput").ap()
    wp_d = nc.dram_tensor("wp_s", (HPC * HS, C), bf16, kind="ExternalInput").ap()
    y_d = nc.dram_tensor("y", (T, C), bf16, kind="ExternalOutput").ap()

    scale = float(1.0 / np.sqrt(HS))

    with tile.TileContext(nc) as tc, ExitStack() as ctx:
        persist = ctx.enter_context(tc.tile_pool(name="persist", bufs=1))
        work = ctx.enter_context(tc.tile_pool(name="work", bufs=3))
        small = ctx.enter_context(tc.tile_pool(name="small", bufs=2))
        outp = ctx.enter_context(tc.tile_pool(name="outp", bufs=6))
        psp = ctx.enter_context(tc.tile_pool(name="psp", bufs=2, space="PSUM"))
        psaux = ctx.enter_context(tc.tile_pool(name="psaux", bufs=2, space="PSUM"))
        psatt = ctx.enter_context(tc.tile_pool(name="psatt", bufs=2, space="PSUM"))

        # ---- persistent SBUF tensors (all bf16) ----
        xt = [persist.tile([128, T], bf16, tag=f"xt{c}", name=f"xt{c}")
              for c in range(NKC)]
        # per-chunk weight tiles: a reader waits only its own chunk's DMA
        wq_sb = [persist.tile([128, 256], bf16, tag=f"wq{c}", name=f"wq{c}")
                 for c in range(NKC)]
        wk_sb = [persist.tile([128, 256], bf16, tag=f"wk{c}", name=f"wk{c}")
                 for c in range(NKC)]
        wv_sb = [persist.tile([128, 256], bf16, tag=f"wv{c}", name=f"wv{c}")
                 for c in range(NKC)]

        def wqs(c):
            return wq_sb[c]

        def wks(c):
            return wk_sb[c]

        def wvs(c):
            return wv_sb[c]
        wp_sb = persist.tile([128, 2, C], bf16, tag="wp")
        qT = [persist.tile([128, T], bf16, tag=f"qT{p}", name=f"qT{p}") for p in range(2)]
        kT = [persist.tile([128, T], bf16, tag=f"kT{p}", name=f"kT{p}") for p in range(2)]
        # 65 used cols (64 v-dims + ones for the denominator), padded to 128
        # so the AV LDWEIGHTS qualifies for compiler-automatic FWL
        vtc = persist.tile([128, NTS, HPC, 128], bf16, tag="vtc")
        attnT = [persist.tile([128, T], bf16, tag=f"attnT{p}", name=f"attnT{p}")
                 for p in range(2)]
        zeros = persist.tile([128, 512], bf16, tag="zeros")
        ones_t = persist.tile([128, NTS, HPC, 1], bf16, tag="ones")

        # ---- init (gpsimd memset starts earliest after boot; zeros gates
        # the PE warmups so it must be ready ASAP) ----
        nc.gpsimd.memset(zeros, 0.0)
        nc.vector.memset(ones_t, 1.0)
        nc.vector.memset(vtc[:, :, :, 65:128], 0.0)
        nc.vector.tensor_copy(out=vtc[:, :, :, 64:65], in_=ones_t)

        # ---- input DMAs, consumption order, 3 queues ----
        # HWDGE (sync evens / scalar odds): wq+xtJ0 interleaved, wk,
        #   xtJ1-J3 merged per chunk.  SWDGE (gpsimd): wv, wp, wp1lo.
        def _ld_w(eng, dst, src, c):
            eng.dma_start(out=dst[c], in_=src[c * 128:(c + 1) * 128, :])

        def _ld_x(eng, c, J):
            eng.dma_start(out=xt[c][:, 512 * J:512 * J + 512],
                          in_=xT_d[c * 128:(c + 1) * 128, 512 * J:512 * J + 512])

        # gpsimd's SWDGE queue has 8 semaphore slots (vs 4 on the HWDGE
        # queues) -> small weight loads go there; the 8 xtJ0 chunks split
        # across sync/scalar so all 8 are in flight at once
        for c in range(NKC):
            _ld_w(nc.gpsimd, wq_sb, wq_d, c)
        for c in range(NKC):
            _ld_x((nc.sync, nc.scalar)[c % 2], c, 0)
        for c in range(NKC):
            _ld_w(nc.gpsimd, wv_sb, wv_d, c)
        for c in range(NKC):
            _ld_w((nc.sync, nc.scalar)[c % 2], wk_sb, wk_d, c)
        nc.gpsimd.dma_start(out=wp_sb, in_=wp_d.rearrange("(k p) n -> p k n", p=128))
        # xtJ1 fine-grained sync/scalar; xtJ2+3 merged all on sync: keeps
        # scalar free for EXPs and gpsimd free for affine_selects from ~14us
        for c in range(NKC):
            _ld_x((nc.sync, nc.scalar)[c % 2], c, 1)
        for c in range(NKC):
            nc.sync.dma_start(out=xt[c][:, 1024:2048],
                              in_=xT_d[c * 128:(c + 1) * 128, 1024:2048])

        # ---- PE warmup (clock ramp during DMA) ----
        for i in range(NWARM):
            pw = psaux.tile([128, 512], f32, tag="aux", name=f"warm{i}")
            nc.tensor.matmul(pw, lhsT=zeros[:, 0:128], rhs=zeros,
                             start=True, stop=True)

        # ---- heartbeat helpers: keep HAM at 8/8 through stalls ----
        hbn = [0]

        def hb_unit(gate_ap):
            # standalone junk matmul in a fresh aux tile, gated on gate_ap
            hbn[0] += 1
            pw = psaux.tile([128, 512], f32, tag="aux", name=f"hbu{hbn[0]}")
            n = min(512, gate_ap.shape[-1])
            k = gate_ap.shape[0]
            nc.tensor.matmul(pw[:, 0:n], lhsT=zeros[0:k, 0:128],
                             rhs=gate_ap[:, 0:n], start=True, stop=True)

        def hb_in(ps_region, gate_ap, n=128):
            # accumulate-zero into a live accumulation region: numeric no-op
            n = min(n, ps_region.shape[-1], gate_ap.shape[-1])
            nc.tensor.matmul(ps_region[:, 0:n], lhsT=zeros[:, 0:128],
                             rhs=gate_ap[:, 0:n], start=False, stop=False,
                             skip_group_check=True)

        # ---------- compute unit emitters ----------
        # (PSUM->SBUF copies must be on DVE/ACT: GPSIMD cannot access PSUM)
        def _act_copy(out, in_):
            nc.scalar.activation(out=out, in_=in_, func=COPYF)

        class _Cp:
            def __init__(self, fn):
                self.tensor_copy = lambda out, in_: fn(out=out, in_=in_)

        cp_rot = [nc.vector, _Cp(_act_copy)]

        def qk_chain(pair, dst, w_of, J, eng_i, hbw=0):
            ps = psaux.tile([128, 512], f32, tag="aux", name=f"qk{id(dst)}_{pair}_{J}")
            for c in range(NKC):
                nc.tensor.matmul(ps, lhsT=w_of(c)[:, 128 * pair:128 * pair + 128],
                                 rhs=xt[c][:, 512 * J:512 * J + 512],
                                 start=(c == 0), stop=(c == NKC - 1))
                if hbw and 0 < c < NKC - 1:
                    for _ in range(hbw):
                        hb_in(ps, w_of(c))
            cp_rot[eng_i % 2].tensor_copy(out=dst[:, 512 * J:512 * J + 512], in_=ps)

        def v_chain(t, hbw=0):
            ps = psaux.tile([128, 512], f32, tag="aux", name=f"v_{t}")
            for c in range(NKC):
                nc.tensor.matmul(ps[:, 0:256], lhsT=xt[c][:, 128 * t:128 * t + 128],
                                 rhs=wvs(c), start=(c == 0), stop=(c == NKC - 1))
                if hbw and 0 < c < NKC - 1:
                    hb_in(ps[:, 0:256], wvs(c))
            cp_rot[t % 2].tensor_copy(
                out=vtc[:, t, :, 0:64],
                in_=ps[:, 0:256].rearrange("p (h x) -> p h x", x=64))

        tail_mode = [False]    # after last att block, proj can use the psp pool

        def proj_tile(m, n):
            if tail_mode[0] and m >= 12:
                py = psp.tile([128, 1024], f32, tag="s",
                              name=f"y_{m}_{n}")[:, 0:512]
            else:
                py = psaux.tile([128, 512], f32, tag="aux", name=f"y_{m}_{n}")
            for pair in range(2):
                nc.tensor.matmul(py, lhsT=attnT[pair][:, 128 * m:128 * m + 128],
                                 rhs=wp_sb[:, pair, 512 * n:512 * n + 512],
                                 start=(pair == 0), stop=(pair == 1))
            yo = outp.tile([128, 512], bf16, tag="yo")
            if tail_mode[0] and (m + n) % 2 == 0:
                # ACT is mostly idle in the tail; alternate with vector
                nc.scalar.activation(out=yo, in_=py, func=COPYF)
            else:
                nc.vector.tensor_copy(out=yo, in_=py)
            nc.sync.dma_start(out=y_d[128 * m:128 * m + 128, 512 * n:512 * n + 512],
                              in_=yo)

        # ---------- filler machinery ----------
        filler = []          # deque of (key, closure)
        emitted = set()

        # 8 units bridge the final normalize chain.  Measured both ways:
        # RESERVE=4 feeds (1,3)'s per-u bubbles but under-covers the
        # normalize window (a 4.3us gap appears at its end) and nets +4us;
        # the full 8 is the right trade.
        RESERVE = 8

        def pull(n):
            for _ in range(n):
                if filler and (tail_mode[0] or len(filler) > RESERVE):
                    k, f = filler.pop(0)
                    f()
                    emitted.add(k)

        def need(*keys):
            # selective: emit only the required units, leave the rest queued
            for k in keys:
                if k in emitted:
                    continue
                for idx, (fk, f) in enumerate(filler):
                    if fk == k:
                        filler.pop(idx)
                        f()
                        emitted.add(k)
                        break
                else:
                    raise RuntimeError(f"missing filler {k}")

        # ---------- attention block pair (both heads interleaved) ----------
        defer = []     # deferred finishers (prev block's normalize, proj adds)

        def att_pair(pair, J, pulls=1, last=False, split_exp=False):
            nch = 4 * J + 4
            # u-iterations: [(t, ss_off, N, qoff, pa_off, diag_stride), ...] x <=2
            us = []
            for i in range(2 * J):
                t0, t1 = 2 * i, 2 * i + 1
                us.append([(t0, 0, 512, 0, 0, 0), (t1, 512, 512, 0, 0, 0)])
            d0 = 4 * J
            us.append([(d0, 0, 512, 0, 0, 512), (d0 + 1, 512, 384, 128, 128, 512)])
            us.append([(d0 + 2, 0, 256, 256, 256, 256),
                       (d0 + 3, 256, 128, 384, 384, 256)])

            pa = [psatt.tile([128, 512], f32, tag="att", name=f"pa_{2*pair+hh}_{J}")
                  for hh in range(2)]
            pend = None
            for ui, u in enumerate(us):
                used = u[-1][1] + u[-1][2]
                dstr = u[0][5]
                et = work.tile([128, 2048], bf16, tag="et", bufs=3)
                for hh in range(2):
                    ss = psp.tile([128, 1024], f32, tag="s",
                                  name=f"ss_{2*pair+hh}_{J}_{u[0][0]}")
                    for (t, off, N, qoff, paoff, _) in u:
                        # K=64 contraction on the head's own partition range
                        nc.tensor.matmul(
                            ss[:, off:off + N],
                            lhsT=kT[pair][64 * hh:64 * hh + 64, 128 * t:128 * t + 128],
                            rhs=qT[pair][64 * hh:64 * hh + 64,
                                         512 * J + qoff:512 * J + qoff + N],
                            start=True, stop=True)
                    if split_exp and used > 512:
                        # early blocks have no proj fillers to hide the
                        # scores->exp->AV fill latency: halving the exp lets
                        # the first AV chunk start one half-exp earlier
                        nc.scalar.activation(
                            out=et[:, 1024 * hh:1024 * hh + 512],
                            in_=ss[:, 0:512], func=EXP, scale=scale)
                        nc.scalar.activation(
                            out=et[:, 1024 * hh + 512:1024 * hh + used],
                            in_=ss[:, 512:used], func=EXP, scale=scale)
                    else:
                        nc.scalar.activation(
                            out=et[:, 1024 * hh:1024 * hh + used],
                            in_=ss[:, 0:used], func=EXP, scale=scale)
                if dstr == 512:
                    # 4 causal triangles (2 heads x 2 diag chunks) evenly
                    # strided, in ONE gpsimd op.  A per-head split (finer AV
                    # gating) was measured at +4us: the doubled gpsimd
                    # instruction count costs more in fixed per-op overhead
                    # on the contended gpsimd queue than the refinement saves.
                    sl = et.rearrange("p (d e) -> p d e", d=4)[:, :, 0:128]
                    nc.gpsimd.affine_select(
                        out=sl, in_=sl, compare_op=mybir.AluOpType.is_ge,
                        fill=0.0, base=0,
                        pattern=[[0, 4], [1, 128]], channel_multiplier=-1)
                elif dstr == 256:
                    for hh in range(2):
                        sl = et[:, 1024 * hh:1024 * hh + 512]
                        sl = sl.rearrange("p (d e) -> p d e", d=2)[:, :, 0:128]
                        nc.gpsimd.affine_select(
                            out=sl, in_=sl, compare_op=mybir.AluOpType.is_ge,
                            fill=0.0, base=0,
                            pattern=[[0, 2], [1, 128]], channel_multiplier=-1)
                if pend is not None:
                    pet, pu = pend
                    for hh in range(2):
                        for (t, off, N, qoff, paoff, _) in pu:
                            nc.tensor.matmul(pa[hh][:, paoff:paoff + N],
                                             lhsT=vtc[:, t, 2 * pair + hh, :],
                                             rhs=pet[:, 1024 * hh + off:1024 * hh + off + N],
                                             start=(t == 0), stop=False)
                pend = (et, u)
                pull(pulls)
                if ui == 0:
                    for fin in defer:
                        fin()
                    defer.clear()
            pet, pu = pend
            for hh in range(2):
                for (t, off, N, qoff, paoff, _) in pu:
                    nc.tensor.matmul(pa[hh][:, paoff:paoff + N],
                                     lhsT=vtc[:, t, 2 * pair + hh, :],
                                     rhs=pet[:, 1024 * hh + off:1024 * hh + off + N],
                                     start=(t == 0), stop=(t == nch - 1))

            # free the pa PSUM tiles immediately: copy denom row + value block
            # to SBUF now; the recip/broadcast/mul chain is deferred into the
            # next block so it never sits ahead of its critical gpsimd ops.
            # The last block has no successor: read pa directly and pipeline
            # vector/gpsimd; the reserved real proj tiles emitted right after
            # the chain keep the PE busy through it.
            if last:
                def finish():
                    s1 = small.tile([1, 512], f32, tag="lsums1")
                    nc.vector.tensor_copy(out=s1, in_=pa[1][64:65, :])
                    s0 = small.tile([1, 512], f32, tag="lsums0")
                    nc.vector.tensor_copy(out=s0, in_=pa[0][64:65, :])
                    rs1 = small.tile([1, 512], f32, tag="rsum")
                    nc.vector.reciprocal_approx_fast(out=rs1, in_=s1)
                    rs0 = small.tile([1, 512], f32, tag="rsum")
                    nc.vector.reciprocal_approx_fast(out=rs0, in_=s0)
                    rb1 = small.tile([64, 512], f32, tag="recip")
                    nc.gpsimd.partition_broadcast(rb1, rs1)
                    tmp = small.tile([64, 512], bf16, tag="tmp")
                    nc.vector.tensor_mul(tmp, pa[1][0:64, :], rb1)
                    rb0 = small.tile([64, 512], f32, tag="recip")
                    nc.gpsimd.partition_broadcast(rb0, rs0)
                    nc.gpsimd.dma_start(
                        out=attnT[pair][64:128, 512 * J:512 * J + 512], in_=tmp)
                    nc.vector.tensor_mul(
                        attnT[pair][0:64, 512 * J:512 * J + 512],
                        pa[0][0:64, :], rb0)
                return finish

            sums = [None, None]
            acc = [None, None]
            for hh in (1, 0):
                sums[hh] = small.tile([1, 512], f32, tag=f"sums{hh}",
                                      name=f"sums{hh}_{pair}_{J}")
                nc.vector.tensor_copy(out=sums[hh], in_=pa[hh][64:65, :])
                acc[hh] = small.tile([64, 512], f32, tag=f"acc{hh}",
                                     name=f"acc{hh}_{pair}_{J}")
                nc.vector.tensor_copy(out=acc[hh], in_=pa[hh][0:64, :])

            def finish():
                for hh in (1, 0):
                    rs = small.tile([1, 512], f32, tag="rsum")
                    nc.vector.reciprocal_approx_fast(out=rs, in_=sums[hh])
                    recip = small.tile([64, 512], f32, tag="recip")
                    nc.gpsimd.partition_broadcast(recip, rs)
                    if hh == 0:
                        nc.vector.tensor_mul(
                            attnT[pair][0:64, 512 * J:512 * J + 512],
                            acc[hh], recip)
                    else:
                        tmp = small.tile([64, 512], bf16, tag="tmp")
                        nc.vector.tensor_mul(tmp, acc[hh], recip)
                        nc.gpsimd.dma_start(
                            out=attnT[pair][64:128, 512 * J:512 * J + 512], in_=tmp)
            return finish

        # ---------- phase A: minimal serial prologue ----------
        # heartbeat-dense: hb_in fires right after each consumed chunk so the
        # PE never idles >~0.5us while DMA paces the first chains
        # v0-3 are NOT serial prologue: they go to the filler-queue front and
        # are pulled inside att(0,0) right after its first scores (pulls=4),
        # overlapping the v chains with score/exp of the first block.  The
        # first AV of (0,0) touches chunks t0,t1 and is emitted after u0's
        # pull, so dependency order stays correct.
        qk_chain(0, qT[0], wqs, 0, 0, hbw=2)
        hb_unit(xt[6][:, 0:512])
        hb_unit(xt[7][:, 0:512])
        qk_chain(0, kT[0], wks, 0, 1, hbw=2)
        hb_unit(wks(5))
        hb_unit(wks(7))

        # ---------- fillers in consumption order ----------
        def qk_unit(pair, dst, w_sb, J, key, eng_i):
            filler.append((key, lambda: qk_chain(pair, dst, w_sb, J, eng_i)))

        # sure-ready warmup units keep the PE clock ramped through the
        # DMA-bound early region
        def warm_unit(i):
            pw = psaux.tile([128, 512], f32, tag="aux", name=f"wf{i}")
            nc.tensor.matmul(pw, lhsT=zeros[:, 0:128], rhs=zeros,
                             start=True, stop=True)

        # v0-3 at the filler front: pulled by att(0,0)'s u0 (pulls=4) so they
        # run after the first scores instead of serializing the prologue
        for t in range(4):
            filler.append((("v", t), lambda t=t: v_chain(t)))
        for i in range(4):
            filler.append((("w", i), lambda i=i: warm_unit(i)))

        ei = 0
        for grp in range(1, NJ):
            # q0/k0 at J=grp ; q1/k1 at J=grp-1 ; v chunks for t-range
            qk_unit(0, qT[0], wqs, grp, ("q", 0, grp), ei); ei += 1
            qk_unit(0, kT[0], wks, grp, ("k", 0, grp), ei); ei += 1
            for t in range(4 * grp, 4 * grp + 4):
                filler.append((("v", t), lambda t=t: v_chain(t)))
            qk_unit(1, qT[1], wqs, grp - 1, ("q", 1, grp - 1), ei); ei += 1
            qk_unit(1, kT[1], wks, grp - 1, ("k", 1, grp - 1), ei); ei += 1
        qk_unit(1, qT[1], wqs, 3, ("q", 1, 3), ei); ei += 1
        qk_unit(1, kT[1], wks, 3, ("k", 1, 3), ei); ei += 1

        def add_proj(J):
            for m in range(4 * J, 4 * J + 4):
                for n in range(2):
                    filler.append(
                        (("proj", m, n), lambda m=m, n=n: proj_tile(m, n)))

        # ---------- merged attention schedule ----------
        def dep_att(pair, J):
            ks = [("q", pair, J), ("k", pair, J)] if (pair, J) != (0, 0) else []
            ks += [("v", t) for t in range(4, 4 * J + 4)]
            return ks

        order = [(0, 0), (0, 1), (1, 0), (0, 2), (1, 1), (0, 3), (1, 2), (1, 3)]
        for (pair, J) in order:
            need(*dep_att(pair, J))
            # split_exp only for the first three blocks (no proj fillers yet
            # to hide the scores->exp->AV fill).  Splitting (1,3)'s exps was
            # measured at +9us: the extra ACT instructions push its final AV
            # and the whole normalize window out.
            fin = att_pair(pair, J,
                           pulls={(0, 0): 4, (1, 0): 2}.get((pair, J), 1),
                           last=((pair, J) == order[-1]),
                           split_exp=((pair, J) in order[:3]))
            defer.append(fin)
            if pair == 1:
                defer.append(lambda J=J: add_proj(J))
        tail_mode[0] = True
        for fin in defer:
            fin()
        defer.clear()
        pull(len(filler))

    nc.compile()
    _NC_CACHE["nc"] = nc
    return nc


def make_in_maps(x, wq, wk, wv, wproj):
    import ml_dtypes
    bf = ml_dtypes.bfloat16
    xTs = [np.ascontiguousarray(x[b].T).astype(bf) for b in range(B)]
    in_maps = []
    for core in range(NCORES):
        b, g = divmod(core, 4)
        hs = slice(4 * g, 4 * g + 4)
        in_maps.append({
            "xT": xTs[b],
            "wq_s": np.ascontiguousarray(
                wq[hs].transpose(1, 0, 2).reshape(C, HPC * HS)).astype(bf),
            "wk_s": np.ascontiguousarray(
                wk[hs].transpose(1, 0, 2).reshape(C, HPC * HS)).astype(bf),
            "wv_s": np.ascontiguousarray(
                wv[hs].transpose(1, 0, 2).reshape(C, HPC * HS)).astype(bf),
            "wp_s": np.ascontiguousarray(
                wproj[4 * g * HS:(4 * g + 4) * HS, :]).astype(bf),
        })
    return in_maps


def _assemble(results, bproj):
    y = np.zeros((B, T, C), dtype=np.float32)
    for core in range(NCORES):
        y[core // 4] += results[core]["y"].astype(np.float32)
    y += bproj.astype(np.float32)[None, None, :]
    return y


def _is_causal(attention_mask):
    tril = np.tril(np.ones((T, T), dtype=bool))
    return all(np.array_equal(attention_mask[b], tril) for b in range(B))


def _numpy_fallback(x, attention_mask, wq, wk, wv, wproj, bproj):
    x64 = x.astype(np.float32)
    q = np.einsum('btc,hcd->bhtd', x64, wq)
    k = np.einsum('btc,hcd->bhtd', x64, wk)
    v = np.einsum('btc,hcd->bhtd', x64, wv)
    wei = np.einsum('bhtd,bhsd->bhts', q, k) / np.sqrt(np.float32(HS))
    wei = np.where(attention_mask[:, None, :, :], wei, -np.inf)
    wei = wei - wei.max(axis=-1, keepdims=True)
    wei = np.exp(wei)
    wei = wei / wei.sum(axis=-1, keepdims=True)
    out = np.einsum('bhts,bhsd->bhtd', wei, v)
    out = out.transpose(0, 2, 1, 3).reshape(B, T, H * HS)
    return (out @ wproj + bproj).astype(np.float32)


def _install_ntff_hook():
    """Recreate the antenv.axon_hooks shim so trace=True works under axon."""
    import sys, types
    try:
        from antenv.axon_hooks import get_axon_ntff_profile_hook  # noqa
        return
    except ImportError:
        pass
    import antenv
    mod = types.ModuleType("antenv.axon_hooks")
    holder = [None]
    mod.set_axon_ntff_profile_hook = lambda h: holder.__setitem__(0, h)
    mod.get_axon_ntff_profile_hook = lambda: holder[0]
    sys.modules["antenv.axon_hooks"] = mod
    antenv.axon_hooks = mod
    if "/root/.axon_site" not in sys.path:
        sys.path.insert(0, "/root/.axon_site")
    from trn_agent_boot.trn_boot import _ntff_profile_via_ctypes
    mod.set_axon_ntff_profile_hook(_ntff_profile_via_ctypes("/opt/axon/libaxon_pjrt.so"))


def kernel(x, attention_mask, wq, wk, wv, wproj, bproj, _trace=False):
    x = np.asarray(x); attention_mask = np.asarray(attention_mask)
    wq = np.asarray(wq); wk = np.asarray(wk); wv = np.asarray(wv)
    wproj = np.asarray(wproj); bproj = np.asarray(bproj)

    if not _is_causal(attention_mask):
        return _numpy_fallback(x, attention_mask, wq, wk, wv, wproj, bproj)

    from concourse import bass_utils
    if _trace:
        _install_ntff_hook()
        bass_utils.upload_artifacts = lambda d: d
    nc = _build_nc()
    in_maps = make_in_maps(x, wq, wk, wv, wproj)
    res = bass_utils.run_bass_kernel_spmd(
        nc, in_maps, core_ids=list(range(NCORES)), trace=_trace)
    out = _assemble(res.results, bproj)
    if _trace:
        return out, res
    return out
